# revision 40
# baseline (speedup 1.0000x reference)
"""Trainium2 Bass kernel for nn_Model_26070451487349 (Informer, sparse_attention).

Strategy: data-parallel over batch (B=8) across 8 NeuronCores; each core runs
the full Informer forward for one batch element. Activations live in SBUF
"d-major" layout [D partitions, L free]. ProbSparse attention top-k/gather/
scatter are done with full-score matmuls + host-precomputed constant masks
(sample-count matrix, sampled-position mask, triangular matrices); the top-k
set is computed with exact-fp32 rank counting (count of strictly-greater M
values + index tie-break), matching jax.lax.top_k semantics.

Matmul operands are fp16 (PE runs 1 cycle/col at any N; ~11-bit mantissa,
same class as f32r but half the memory); PSUM accumulation is fp32 and all
top-k comparisons are exact fp32.
"""
import sys
import os
import numpy as np
from contextlib import ExitStack

for _p in ("/opt/trn_rl_repo",):
    if _p not in sys.path:
        sys.path.insert(0, _p)

import concourse.bass as bass
import concourse.tile as tile
from concourse import bass_utils, bacc, mybir

F32 = mybir.dt.float32
F16 = mybir.dt.float16
AL = mybir.AluOpType
AF = mybir.ActivationFunctionType
AX = mybir.AxisListType

# ---- model dims (hardcoded from reference) ----
B = 8
SEQ = 720          # encoder length
DEC = 672          # decoder length
PRED = 336
CIN = 321
CINP = 384         # padded to 3x128
COUT = 321
MARK = 4
D = 512
H = 8
DH = 64
DFF = 2048
FACTOR = 5
EPS = 1e-5
NEG = -60000.0     # "-inf" for fp16 sampled-mask

# attention specs: (LQ, LK, u, Upart, fold, masked)
ATTN_SPECS = {
    'e0': dict(LQ=720, LK=720, u=35, Up=35, fold=0, masked=False),
    'e1': dict(LQ=360, LK=360, u=30, Up=30, fold=1, masked=False),
    'ds': dict(LQ=672, LK=672, u=35, Up=35, fold=100, masked=True),
    'dc': dict(LQ=672, LK=360, u=35, Up=30, fold=200, masked=False),
}


def _pt(L):
    # partition-tile size for a sequence length
    return 120 if L % 120 == 0 else 112


def _halves(L):
    if L <= 512:
        return [(0, L)]
    h = L // 2
    return [(0, h), (h, L - h)]


# ---------------------------------------------------------------------------
# host-side constants
# ---------------------------------------------------------------------------

def _pos_embedding_np(L, d):
    pos = np.arange(L, dtype=np.float32)[:, None]
    div = np.exp(np.arange(0, d, 2, dtype=np.float32) * -(np.log(10000.0) / d))
    pe = np.zeros((L, d), dtype=np.float32)
    pe[:, 0::2] = np.sin(pos * div)
    pe[:, 1::2] = np.cos(pos * div)
    return pe


def _tile_lm(a, PT):
    # [L, K] -> [PT, L//PT, K]  (partition-tiled rows)
    L, Kd = a.shape
    n = L // PT
    return np.ascontiguousarray(a.reshape(n, PT, Kd).transpose(1, 0, 2))


def _dmaj(a):
    # [Din, Dout] -> [128, Din//128, Dout]
    Din, Dout = a.shape
    return np.ascontiguousarray(a.reshape(Din // 128, 128, Dout).transpose(1, 0, 2))


_CONSTS = None


def _host_constants():
    global _CONSTS
    if _CONSTS is not None:
        return _CONSTS
    import jax
    cpu = jax.devices('cpu')[0]
    c = {}
    iota = np.tile(np.arange(768, dtype=np.float32), (128, 1))
    c['iota'] = iota[:, :720].copy()
    c['ident'] = np.eye(128, dtype=np.float16)
    c['ident32'] = np.eye(128, dtype=np.float32)
    c['ones'] = np.ones((128, 128), dtype=np.float16)
    c['ones32'] = np.ones((1, 128), dtype=np.float32)
    with jax.default_device(cpu):
        k0 = jax.random.key(42)
        for name, sp in ATTN_SPECS.items():
            LQ, LK, u, Up = sp['LQ'], sp['LK'], sp['u'], sp['Up']
            key = jax.random.fold_in(k0, sp['fold'])
            idx = np.array(jax.random.randint(key, (LQ, Up), 0, LK))
            cm = np.zeros((LQ, LK), np.float32)
            np.add.at(cm, (np.arange(LQ)[:, None], idx), 1.0)
            nm = np.where(cm > 0, 0.0, NEG).astype(np.float32)
            PT = _pt(LQ)
            c[f'cm_{name}'] = _tile_lm(cm, PT).astype(np.float16)
            c[f'nm_{name}'] = _tile_lm(nm, PT).astype(np.float16)
            lt = np.tril(np.ones((LQ, LQ), np.float32), -1)
            c[f'lt_{name}'] = _tile_lm(lt, PT).astype(np.float16)
            if sp['masked']:
                ut = (np.arange(LQ)[:, None] <= np.arange(LQ)[None, :]).astype(np.float32)
                c[f'ut_{name}'] = _tile_lm(ut, PT).astype(np.float16)
                ic = np.arange(LQ, dtype=np.float32).reshape(LQ, 1)
                c[f'ic_{name}'] = _tile_lm(ic, PT).astype(np.float16)
    pe = _pos_embedding_np(SEQ, D)     # [720, 512]
    c['pos_e'] = _dmaj(pe.T.copy().astype(np.float32)).astype(np.float16)
    pd = _pos_embedding_np(DEC, D)
    c['pos_d'] = _dmaj(pd.T.copy().astype(np.float32))
    _CONSTS = c
    return c


def _f16(a):
    return np.ascontiguousarray(a).astype(np.float16)


def _bcol(b, Mc):
    # bias [Dout] -> [128, Mc] (column m holds bias of m-th 128-chunk), fp32
    bp = np.zeros(Mc * 128, np.float32)
    bp[:b.shape[0]] = b
    return np.ascontiguousarray(bp.reshape(Mc, 128).T)


def _pack_params(params):
    p = {k: {kk: np.asarray(vv, np.float32) if not isinstance(vv, (dict, list)) else vv
             for kk, vv in v.items()} if isinstance(v, dict) else v
         for k, v in params.items()}

    w = {}

    def attn_w(pref, a):
        for nm in ('wq', 'wk', 'wv', 'wo'):
            w[f'{pref}{nm}'] = _f16(_dmaj(np.asarray(a[nm], np.float32)))
        for nm in ('bq', 'bk', 'bo'):
            w[f'{pref}{nm}'] = _bcol(np.asarray(a[nm], np.float32), 4)
        w[f'{pref}bvr'] = _f16(np.asarray(a['bv'], np.float32).reshape(1, D))

    def ffn_w(pref, a):
        w[f'{pref}w1'] = _f16(_dmaj(np.asarray(a['w1'], np.float32)))
        w[f'{pref}w2'] = _f16(_dmaj(np.asarray(a['w2'], np.float32)))
        w[f'{pref}b1'] = _bcol(np.asarray(a['b1'], np.float32), 16)
        w[f'{pref}b2'] = _bcol(np.asarray(a['b2'], np.float32), 4)

    def ln_w(pref, a):
        w[f'{pref}g'] = _bcol(np.asarray(a['g'], np.float32), 4)
        w[f'{pref}b'] = _bcol(np.asarray(a['b'], np.float32), 4)

    def conv_w(pref, cw, cinp, lo=False):
        # cw [Dout, Cin, 3] -> [128, cinp//128, 3, Dout] fp16 (+ optional lo part)
        cw = np.asarray(cw, np.float32)
        Dout, Cin, _ = cw.shape
        a = np.zeros((3, cinp, Dout), np.float32)
        for k in range(3):
            a[k, :Cin, :] = cw[:, :, k].T
        cinc = cinp // 128
        a = a.reshape(3, cinc, 128, Dout).transpose(2, 1, 0, 3)
        hi = a.astype(np.float16)
        w[pref] = hi
        if lo:
            w[pref + 'l'] = (a - hi.astype(np.float32)).astype(np.float16)

    conv_w('cwe', p['enc_emb']['conv_w'], CINP)
    w['mwe'] = _f16(np.asarray(p['enc_emb']['mark_w'], np.float32))
    conv_w('cwd2', p['dec_emb']['conv_w'], CINP, lo=True)
    w['mwd'] = _f16(np.asarray(p['dec_emb']['mark_w'], np.float32))

    e0, e1 = p['enc_layers'][0], p['enc_layers'][1]
    attn_w('e0', e0['attn']); ffn_w('e0', e0); ln_w('e0n1', e0['norm1']); ln_w('e0n2', e0['norm2'])
    attn_w('e1', e1['attn']); ffn_w('e1', e1); ln_w('e1n1', e1['norm1']); ln_w('e1n2', e1['norm2'])

    cv = p['conv_layers'][0]
    s = np.asarray(cv['bn_g'], np.float32) / np.sqrt(np.float32(1.0) + np.float32(EPS))
    cwd = np.asarray(cv['w'], np.float32) * s[:, None, None]
    conv_w('cwd', cwd, D)
    w['bd'] = _bcol(np.asarray(cv['b'], np.float32) * s + np.asarray(cv['bn_b'], np.float32), 4)

    ln_w('encn', p['enc_norm'])

    dl = p['dec_layers'][0]
    attn_w('ds', dl['self_attn'])
    # hi/lo split for fp32-quality Q/K projections in the masked attention
    for nm in ('wq', 'wk'):
        wf = np.asarray(dl['self_attn'][nm], np.float32)
        hi = wf.astype(np.float16)
        lo = (wf - hi.astype(np.float32)).astype(np.float16)
        w[f'ds{nm}'] = _f16(_dmaj(hi.astype(np.float32)))
        w[f'ds{nm}l'] = _f16(_dmaj(lo.astype(np.float32)))
    attn_w('dc', dl['cross_attn'])
    ffn_w('d', dl)
    ln_w('dn1', dl['norm1']); ln_w('dn2', dl['norm2']); ln_w('dn3', dl['norm3'])
    ln_w('decn', p['dec_norm'])

    pw = np.zeros((D, CINP), np.float32)
    pw[:, :COUT] = np.asarray(p['proj_w'], np.float32)
    w['pw'] = _f16(_dmaj(pw))
    w['pb'] = _bcol(np.asarray(p['proj_b'], np.float32), 3)
    return w


# ---------------------------------------------------------------------------
# kernel builder
# ---------------------------------------------------------------------------

STOP_AFTER = os.environ.get('KSTOP', 'all')   # embed|enc0|distill|enc1|dembed|ds|dc|all
DEBUG = bool(int(os.environ.get('KDEBUG', '0')))


class KB:
    def __init__(self, nc, tc, ctx, dram):
        self.nc = nc
        self.tc = tc
        self.dram = dram
        p = lambda name, bufs: ctx.enter_context(tc.tile_pool(name=name, bufs=bufs))
        self.cpool = p("const", 1)
        self.wpool = p("wstream", 2)
        self.mpool = p("mask", 1)
        self.apool = p("act", 1)      # large per-layer activations (distinct tags)
        self.rpool = p("resid", 2)
        self.spool = p("scr", 2)      # [<=128, <=720] fp32 scratch
        self.kpool = p("small", 3)    # [<=128, <=64] small fp32 tiles
        self.bigp = p("big", 1)       # shared slot: ffn-hidden / conv weights / UT
        self.psum_n = ctx.enter_context(tc.tile_pool(name="psum_n", bufs=4, space="PSUM"))
        self.psum_s = ctx.enter_context(tc.tile_pool(name="psum_s", bufs=2, space="PSUM"))
        self.psum_m = ctx.enter_context(tc.tile_pool(name="psum_m", bufs=2, space="PSUM"))
        self.dbg = {}

        # persistent consts
        self.iota = self.ld('iota', [128, 720], F32, pool=self.cpool)
        self.ident = self.ld('ident', [128, 128], F16, pool=self.cpool)
        self.ident32 = self.ld('ident32', [128, 128], F32, pool=self.cpool)
        self.ones = self.ld('ones', [128, 128], F16, pool=self.cpool)
        self.ones32 = self.ld('ones32', [1, 128], F32, pool=self.cpool)
        self.epsc = self.cpool.tile([128, 1], F32, tag="epsc")
        nc.vector.memset(self.epsc[:], EPS)

    # -- misc helpers --
    def ld(self, name, shape, dt, pool=None, tag=None):
        pool = pool or self.cpool
        t = pool.tile(shape, dt, tag=tag or name)
        ap = self.dram[name].ap()
        self.nc.sync.dma_start(out=t[:], in_=ap)
        return t

    def dump(self, name, ap_or_tile, shape, dt=F32):
        if not DEBUG:
            return
        d = self.nc.dram_tensor(f'dbg_{name}', shape, dt, kind="ExternalOutput")
        self.nc.sync.dma_start(out=d.ap(), in_=ap_or_tile)
        self.dbg[name] = shape

    def dump_dmaj(self, name, X, Mc, L):
        # dump a [128, Mc, L] tile as [Mc*128, L] fp32 dram
        if not DEBUG:
            return
        d = self.nc.dram_tensor(f'dbg_{name}', [Mc * 128, L], F32, kind="ExternalOutput")
        for m in range(Mc):
            s = self.spool.tile([128, L], F32, tag="dmp")
            self.nc.scalar.copy(s[:], X[:, m, :])
            self.nc.sync.dma_start(out=d.ap().rearrange("(mc p) l -> p mc l", p=128)[:, m, :], in_=s[:])

    # -- building blocks --

    def lin_dmaj(self, X, wname, L, Mc=4, Kc=4, evict=None, bname=None, act=AF.Identity,
                 out=None, out_tag=None, odt=F16, wlo=None, xlo=None):
        """Y[., m, l] = act(sum_k W[k, m-chunk].T X[k, l] + b).  X: [128, Kc, L] fp16.
        Streams weights in m-groups of 4 chunks. wlo: optional low-half weight
        tensor (fp32-quality weights via hi+lo fp16 matmuls)."""
        nc = self.nc
        if out is None:
            out = self.apool.tile([128, Mc, L], odt, tag=out_tag or wname[:2] + "_o")
        wd = self.dram[wname]
        bt = None
        if bname is not None:
            bt = self.ld(bname, [128, Mc], F32, pool=self.cpool)
        for g in range((Mc + 3) // 4):
            mg = min(4, Mc - g * 4)
            wt = self.wpool.tile([128, Kc, mg * 128], F16, tag="wstream")
            nc.sync.dma_start(out=wt[:], in_=wd.ap()[:, :, g * 512:g * 512 + mg * 128])
            wt2 = None
            if wlo is not None:
                wt2 = self.wpool.tile([128, Kc, mg * 128], F16, tag="wstream")
                nc.sync.dma_start(out=wt2[:], in_=self.dram[wlo].ap()[:, :, g * 512:g * 512 + mg * 128])
            for mm in range(mg):
                m = g * 4 + mm
                for (h0, nn) in _halves(L):
                    ps = self.psum_n.tile([128, nn], F32, tag="pn")
                    last = (wt2 is None and xlo is None)
                    for kc in range(Kc):
                        nc.tensor.matmul(ps[:], wt[:, kc, mm * 128:(mm + 1) * 128],
                                         X[:, kc, h0:h0 + nn],
                                         start=(kc == 0), stop=(kc == Kc - 1 and last))
                    if xlo is not None:
                        for kc in range(Kc):
                            nc.tensor.matmul(ps[:], wt[:, kc, mm * 128:(mm + 1) * 128],
                                             xlo[:, kc, h0:h0 + nn],
                                             start=False, stop=(kc == Kc - 1 and wt2 is None))
                    if wt2 is not None:
                        for kc in range(Kc):
                            nc.tensor.matmul(ps[:], wt2[:, kc, mm * 128:(mm + 1) * 128],
                                             X[:, kc, h0:h0 + nn],
                                             start=False, stop=(kc == Kc - 1))
                    if evict is not None:
                        evict(ps, m, h0, nn, out)
                    elif bt is not None:
                        nc.scalar.activation(out[:, m, h0:h0 + nn], ps[:], act,
                                             bias=bt[:, m:m + 1], scale=1.0)
                    else:
                        nc.scalar.activation(out[:, m, h0:h0 + nn], ps[:], act)
        return out

    def lin_tokmaj(self, X, wname, brname, L, PT, NT, Kc=4, out_tag=None):
        """V[lt-part, lt, dout] = X^T W + b_row. Returns [PT, NT, 512] fp16."""
        nc = self.nc
        out = self.apool.tile([PT, NT, D], F16, tag=out_tag or "vtok")
        wt = self.wpool.tile([128, Kc, D], F16, tag="wstream")
        nc.sync.dma_start(out=wt[:], in_=self.dram[wname].ap())
        br = self.cpool.tile([1, D], F16, tag=brname)
        nc.sync.dma_start(out=br[:], in_=self.dram[brname].ap())
        for lt in range(NT):
            ps = self.psum_n.tile([PT, D], F32, tag="pn")
            for kc in range(Kc):
                nc.tensor.matmul(ps[:], X[:, kc, lt * PT:(lt + 1) * PT], wt[:, kc, :],
                                 start=(kc == 0), stop=False)
            nc.tensor.matmul(ps[:], self.ones[0:1, 0:PT], br[:], start=False, stop=True)
            nc.scalar.copy(out[:, lt, :], ps[:])
        return out

    def layernorm(self, X, gname, bname, L, out_tag="resid"):
        """post-norm layernorm over d (partition dim), d-major layout."""
        nc = self.nc
        g = self.ld(gname, [128, 4], F32, pool=self.cpool)
        b = self.ld(bname, [128, 4], F32, pool=self.cpool)
        pool = self.rpool if out_tag == "resid" else self.apool
        out = pool.tile([128, 4, L], F16, tag=out_tag)
        for (h0, nn) in _halves(L):
            psr = self.psum_m.tile([1, nn], F32, tag="pm")
            for kc in range(4):
                nc.tensor.matmul(psr[:], self.ones[:, 0:1], X[:, kc, h0:h0 + nn],
                                 start=(kc == 0), stop=(kc == 3))
            mu = self.spool.tile([1, nn], F16, tag="murow")
            nc.scalar.activation(mu[:], psr[:], AF.Copy, scale=1.0 / 512.0)
            # broadcast mean and subtract
            mb = self.psum_m.tile([128, nn], F32, tag="pm")
            nc.tensor.matmul(mb[:], self.ones[0:1, :], mu[:], start=True, stop=True)
            xc = self.apool.tile([128, 4, nn], F16, tag="xa")
            sq = self.spool.tile([128, nn], F16, tag="sqscr")
            psv = self.psum_m.tile([1, nn], F32, tag="pm")
            for kc in range(4):
                nc.vector.tensor_tensor(xc[:, kc, :], X[:, kc, h0:h0 + nn], mb[:], op=AL.subtract)
                nc.vector.tensor_tensor(sq[:], xc[:, kc, :], xc[:, kc, :], op=AL.mult)
                nc.tensor.matmul(psv[:], self.ones[:, 0:1], sq[:], start=(kc == 0), stop=(kc == 3))
                sq = self.spool.tile([128, nn], F16, tag="sqscr")
            sd = self.spool.tile([1, nn], F32, tag="sdrow")
            nc.scalar.activation(sd[:], psv[:], AF.Sqrt, bias=self.epsc[0:1, :], scale=1.0 / 512.0)
            rstd = self.spool.tile([1, nn], F16, tag="rstdrow")
            nc.vector.reciprocal(rstd[:], sd[:])
            rb = self.psum_m.tile([128, nn], F32, tag="pm")
            nc.tensor.matmul(rb[:], self.ones[0:1, :], rstd[:], start=True, stop=True)
            for kc in range(4):
                t = self.spool.tile([128, nn], F32, tag="sqscr")
                nc.vector.tensor_tensor(t[:], xc[:, kc, :], rb[:], op=AL.mult)
                nc.scalar.activation(out[:, kc, h0:h0 + nn], t[:], AF.Identity,
                                     bias=b[:, kc:kc + 1], scale=g[:, kc:kc + 1])
        return out

    def ffn(self, X, pref, L):
        """gelu(X W1 + b1) W2 + b2 + X, returns pre-LN residual sum tile."""
        nc = self.nc
        b1 = self.ld(pref + 'b1', [128, 16], F32, pool=self.cpool)
        b2 = self.ld(pref + 'b2', [128, 4], F32, pool=self.cpool)
        out = self.rpool.tile([128, 4, L], F16, tag="resid")
        w1d = self.dram[pref + 'w1']
        w2d = self.dram[pref + 'w2']
        for (h0, nn) in _halves(L):
            ht = self.bigp.tile([128, 16, nn], F16, tag="big")
            for g in range(4):
                wt = self.wpool.tile([128, 4, 512], F16, tag="wstream")
                nc.sync.dma_start(out=wt[:], in_=w1d.ap()[:, :, g * 512:(g + 1) * 512])
                for mm in range(4):
                    mf = g * 4 + mm
                    ps = self.psum_n.tile([128, nn], F32, tag="pn")
                    for kc in range(4):
                        nc.tensor.matmul(ps[:], wt[:, kc, mm * 128:(mm + 1) * 128],
                                         X[:, kc, h0:h0 + nn], start=(kc == 0), stop=(kc == 3))
                    nc.scalar.activation(ht[:, mf, :], ps[:], AF.Gelu,
                                         bias=b1[:, mf:mf + 1], scale=1.0)
            pss = [self.psum_n.tile([128, nn], F32, tag="pn", name=f"pss{i}") for i in range(4)]
            for g in range(4):
                wt = self.wpool.tile([128, 4, 512], F16, tag="wstream")
                nc.sync.dma_start(out=wt[:], in_=w2d.ap()[:, g * 4:(g + 1) * 4, :])
                for m in range(4):
                    for kk in range(4):
                        nc.tensor.matmul(pss[m][:], wt[:, kk, m * 128:(m + 1) * 128],
                                         ht[:, g * 4 + kk, :],
                                         start=(g == 0 and kk == 0), stop=(g == 3 and kk == 3))
            for m in range(4):
                nc.vector.scalar_tensor_tensor(out[:, m, h0:h0 + nn], pss[m][:],
                                               b2[:, m:m + 1], X[:, m, h0:h0 + nn],
                                               op0=AL.add, op1=AL.add)
        return out

    def embed(self, xt, markt, cwname, mwname, posname, L, Cc, xtl=None, want_lo=False):
        """token conv3 (circular) + pos + mark embedding -> [128, 4, L] fp16.
        With xtl/want_lo: hi/lo input+weights for fp32-quality output; returns
        (hi, lo) fp16 pair whose sum is the fp32-accurate embedding."""
        nc = self.nc
        xpad = self.apool.tile([128, Cc, L + 2], F16, tag="xa")
        for c in range(Cc):
            nc.vector.tensor_copy(xpad[:, c, 1:L + 1], xt[:, c, :])
            nc.vector.tensor_copy(xpad[:, c, 0:1], xt[:, c, L - 1:L])
            nc.vector.tensor_copy(xpad[:, c, L + 1:L + 2], xt[:, c, 0:1])
        xpadl = None
        if xtl is not None:
            xpadl = self.apool.tile([128, Cc, L + 2], F16, tag="xal")
            for c in range(Cc):
                nc.vector.tensor_copy(xpadl[:, c, 1:L + 1], xtl[:, c, :])
                nc.vector.tensor_copy(xpadl[:, c, 0:1], xtl[:, c, L - 1:L])
                nc.vector.tensor_copy(xpadl[:, c, L + 1:L + 2], xtl[:, c, 0:1])
        cw = self.bigp.tile([128, Cc, 3, D], F16, tag="big")
        nc.sync.dma_start(out=cw[:], in_=self.dram[cwname].ap())
        cwl = None
        if xtl is not None:
            cwl = self.apool.tile([128, Cc, 3, D], F16, tag="cwl")
            nc.sync.dma_start(out=cwl[:], in_=self.dram[cwname + 'l'].ap())
        mw = self.cpool.tile([MARK, D], F16, tag=mwname)
        nc.sync.dma_start(out=mw[:], in_=self.dram[mwname].ap())
        pdt = F32 if want_lo else F16
        pos = self.ld(posname, [128, 4, L], pdt, pool=self.cpool, tag="pos")
        out = self.rpool.tile([128, 4, L], F16, tag="resid")
        out_lo = self.apool.tile([128, 4, L], F16, tag="xl", name="out_lo") if want_lo else None
        for m in range(4):
            for (h0, nn) in _halves(L):
                ps = self.psum_n.tile([128, nn], F32, tag="pn")
                first = True
                for k in range(3):
                    for c in range(Cc):
                        nc.tensor.matmul(ps[:], cw[:, c, k, m * 128:(m + 1) * 128],
                                         xpad[:, c, k + h0:k + h0 + nn],
                                         start=first, stop=False)
                        first = False
                        if xtl is not None:
                            nc.tensor.matmul(ps[:], cw[:, c, k, m * 128:(m + 1) * 128],
                                             xpadl[:, c, k + h0:k + h0 + nn],
                                             start=False, stop=False)
                            nc.tensor.matmul(ps[:], cwl[:, c, k, m * 128:(m + 1) * 128],
                                             xpad[:, c, k + h0:k + h0 + nn],
                                             start=False, stop=False)
                nc.tensor.matmul(ps[:], mw[:, m * 128:(m + 1) * 128], markt[:, h0:h0 + nn],
                                 start=False, stop=True)
                if want_lo:
                    t = self.spool.tile([128, nn], F32, tag="scr")
                    nc.vector.tensor_tensor(t[:], ps[:], pos[:, m, h0:h0 + nn], op=AL.add)
                    nc.vector.tensor_copy(out[:, m, h0:h0 + nn], t[:])
                    nc.vector.tensor_tensor(out_lo[:, m, h0:h0 + nn], t[:],
                                            out[:, m, h0:h0 + nn], op=AL.subtract)
                else:
                    nc.vector.tensor_tensor(out[:, m, h0:h0 + nn], ps[:], pos[:, m, h0:h0 + nn],
                                            op=AL.add)
        return out, out_lo

    def attention(self, aname, XQ, XKV, LQ, LK, XQlo=None, XKVlo=None):
        """ProbSparse attention. Returns OT d-major [128, 4, LQ] fp16 (pre-Wo)."""
        nc = self.nc
        sp = ATTN_SPECS[aname]
        u, masked = sp['u'], sp['masked']
        PTQ, NQ = _pt(LQ), LQ // _pt(LQ)
        PTK, NK = _pt(LK), LK // _pt(LK)
        hvK = _halves(LK)

        # high-precision Q/K path for the masked attention: its top-k flips
        # swap attention vs cumsum rows (large damage), so Q/K stay fp32 and
        # Wq/Wk (and the input) are applied as hi+lo fp16 pairs.
        hiprec = masked
        qdt = F32 if hiprec else F16
        QT = self.lin_dmaj(XQ, aname + 'wq', LQ, bname=aname + 'bq', out_tag="qt",
                           odt=qdt, wlo=(aname + 'wql') if hiprec else None,
                           xlo=XQlo if hiprec else None)
        KT = self.lin_dmaj(XKV, aname + 'wk', LK, bname=aname + 'bk', out_tag="kt",
                           odt=qdt, wlo=(aname + 'wkl') if hiprec else None,
                           xlo=XKVlo if hiprec else None)
        V = self.lin_tokmaj(XKV, aname + 'wv', aname + 'bvr', LK, PTK, NK)

        cm = self.mpool.tile([PTQ, NQ, LK], F16, tag="cm")
        nc.sync.dma_start(out=cm[:], in_=self.dram['cm_' + aname].ap())
        nm = self.mpool.tile([PTQ, NQ, LK], F16, tag="nm")
        nc.sync.dma_start(out=nm[:], in_=self.dram['nm_' + aname].ap())
        ltm = self.mpool.tile([PTQ, NQ, LQ], F16, tag="ltm")
        nc.sync.dma_start(out=ltm[:], in_=self.dram['lt_' + aname].ap())
        if masked:
            ut = self.bigp.tile([PTQ, NQ, LQ], F16, tag="big")
            nc.sync.dma_start(out=ut[:], in_=self.dram['ut_' + aname].ap())
            ic = self.cpool.tile([PTQ, NQ, 1], F16, tag="ic_" + aname)
            nc.sync.dma_start(out=ic[:], in_=self.dram['ic_' + aname].ap())

        OT = self.apool.tile([128, 4, LQ], F16, tag="ot")

        for h in range(H):
            ti, r0 = (h * DH) // 128, (h * DH) % 128
            # ---- S = q k^T (token-major [lq, lk]); M stats read fp32 psum ----
            s_sb = self.apool.tile([PTQ, NQ, LK], F16, tag="s_sb", bufs=2)
            mcol = self.spool.tile([PTQ, NQ], F32, tag="mcol")
            for lt in range(NQ):
                mxs, sms = [], []
                for (h0, nn) in hvK:
                    ps = self.psum_s.tile([PTQ, nn], F32, tag="psS")
                    nc.tensor.matmul(ps[:], QT[r0:r0 + DH, ti, lt * PTQ:(lt + 1) * PTQ],
                                     KT[r0:r0 + DH, ti, h0:h0 + nn], start=True, stop=True)
                    nc.scalar.copy(s_sb[:, lt, h0:h0 + nn], ps[:])
                    mx = self.kpool.tile([PTQ, 1], F32, tag="mx", name=f"mx{lt}")
                    sm = self.kpool.tile([PTQ, 1], F32, tag="sm", name=f"sm{lt}")
                    if hiprec:
                        scr = self.spool.tile([PTQ, nn], F32, tag="scr")
                        nc.vector.tensor_tensor(scr[:], ps[:], nm[:, lt, h0:h0 + nn], op=AL.add)
                        nc.vector.tensor_reduce(mx[:], scr[:], axis=AX.X, op=AL.max)
                        scr2 = self.spool.tile([PTQ, nn], F32, tag="scr")
                        nc.vector.scalar_tensor_tensor(scr2[:], ps[:], 1.0 / LK, cm[:, lt, h0:h0 + nn],
                                                       op0=AL.mult, op1=AL.mult, accum_out=sm[:])
                    else:
                        scr = self.spool.tile([PTQ, nn], F16, tag="scr")
                        nc.gpsimd.tensor_tensor(scr[:], s_sb[:, lt, h0:h0 + nn],
                                                nm[:, lt, h0:h0 + nn], op=AL.add)
                        nc.vector.tensor_reduce(mx[:], scr[:], axis=AX.X, op=AL.max)
                        scr2 = self.spool.tile([PTQ, nn], F16, tag="scr")
                        nc.vector.scalar_tensor_tensor(scr2[:], s_sb[:, lt, h0:h0 + nn], 1.0 / LK,
                                                       cm[:, lt, h0:h0 + nn],
                                                       op0=AL.mult, op1=AL.mult, accum_out=sm[:])
                    mxs.append(mx)
                    sms.append(sm)
                if len(hvK) == 2:
                    nc.vector.tensor_tensor(mxs[0][:], mxs[0][:], mxs[1][:], op=AL.max)
                    nc.vector.tensor_tensor(sms[0][:], sms[0][:], sms[1][:], op=AL.add)
                nc.vector.tensor_tensor(mcol[:, lt:lt + 1], mxs[0][:], sms[0][:], op=AL.subtract)
            # ---- exact top-u rank ----
            mrow = self.spool.tile([1, LQ], F32, tag="mrow", bufs=1)
            for lt in range(NQ):
                tp = self.psum_m.tile([1, PTQ], F32, tag="pm")
                nc.tensor.matmul(tp[:], mcol[:, lt:lt + 1], self.ident32[0:PTQ, 0:PTQ],
                                 is_transpose=True, start=True, stop=True)
                nc.scalar.copy(mrow[0:1, lt * PTQ:(lt + 1) * PTQ], tp[:])
            hvQ = _halves(LQ)
            mbs = []
            for qi, (q0, qn) in enumerate(hvQ):
                mbp = self.psum_s.tile([PTQ, qn], F32, tag="psS", name=f"mbp{qi}")
                nc.tensor.matmul(mbp[:], self.ones32[0:1, 0:PTQ], mrow[0:1, q0:q0 + qn],
                                 start=True, stop=True)
                mbs.append((mbp, q0, qn))
            sel = self.apool.tile([PTQ, NQ, u], F16, tag="sel")
            ntop = self.spool.tile([PTQ, NQ], F32, tag="ntop")
            for lt in range(NQ):
                rank = self.kpool.tile([PTQ, 1], F32, tag="rank")
                for qi, (mbp, q0, qn) in enumerate(mbs):
                    scr = self.spool.tile([PTQ, qn], F16, tag="scr")
                    cgt = self.kpool.tile([PTQ, 1], F32, tag="cgt", name=f"cgt{qi}")
                    nc.vector.tensor_scalar(scr[:], mbp[:], mcol[:, lt:lt + 1], 0.0,
                                            op0=AL.is_gt, op1=AL.add, accum_out=cgt[:])
                    scr2 = self.spool.tile([PTQ, qn], F16, tag="scr")
                    ceq = self.kpool.tile([PTQ, 1], F32, tag="ceq", name=f"ceq{qi}")
                    nc.vector.scalar_tensor_tensor(scr2[:], mbp[:], mcol[:, lt:lt + 1],
                                                   ltm[:, lt, q0:q0 + qn],
                                                   op0=AL.is_equal, op1=AL.mult, accum_out=ceq[:])
                    if qi == 0:
                        nc.vector.tensor_tensor(rank[:], cgt[:], ceq[:], op=AL.add)
                    else:
                        nc.vector.tensor_tensor(rank[:], rank[:], cgt[:], op=AL.add)
                        nc.vector.tensor_tensor(rank[:], rank[:], ceq[:], op=AL.add)
                nc.vector.tensor_scalar(sel[:, lt, :], self.iota[0:PTQ, 0:u], rank[:], None,
                                        op0=AL.is_equal)
                nc.vector.tensor_scalar(ntop[:, lt:lt + 1], rank[:], float(u) - 0.5, None,
                                        op0=AL.is_ge)
            if DEBUG:
                self.dump(f'{aname}_mcol_h{h}', mcol[:], [PTQ, NQ])
            # ---- scores = Sel^T S  (+ causal mask) -> softmax ----
            if masked:
                tps = self.psum_m.tile([u, 1], F32, tag="pm")
                for lt in range(NQ):
                    nc.tensor.matmul(tps[:], sel[:, lt, :], ic[:, lt, :],
                                     start=(lt == 0), stop=(lt == NQ - 1))
                tcol = self.kpool.tile([u, 1], F32, tag="tcol")
                nc.scalar.copy(tcol[:], tps[:])
            srcs = []
            for (h0, nn) in hvK:
                ps = self.psum_s.tile([u, nn], F32, tag="psS")
                for lt in range(NQ):
                    nc.tensor.matmul(ps[:], sel[:, lt, :], s_sb[:, lt, h0:h0 + nn],
                                     start=(lt == 0), stop=(lt == NQ - 1))
                if masked:
                    madd = self.spool.tile([u, nn], F32, tag="madd")
                    nc.vector.tensor_scalar(madd[:], self.iota[0:u, h0:h0 + nn], tcol[:], -1e30,
                                            op0=AL.is_gt, op1=AL.mult)
                    sc = self.spool.tile([u, nn], F32, tag="scmask")
                    nc.vector.tensor_tensor(sc[:], ps[:], madd[:], op=AL.add)
                    srcs.append((sc, h0, nn))
                else:
                    srcs.append((ps, h0, nn))
            rmxs = []
            for (src, h0, nn) in srcs:
                r = self.kpool.tile([u, 1], F32, tag="rmx")
                nc.vector.tensor_reduce(r[:], src[:], axis=AX.X, op=AL.max)
                rmxs.append(r)
            rmx = rmxs[0]
            if len(rmxs) == 2:
                rmx2 = self.kpool.tile([u, 1], F32, tag="rmx")
                nc.vector.tensor_tensor(rmx2[:], rmxs[0][:], rmxs[1][:], op=AL.max)
                rmx = rmx2
            negmx = self.kpool.tile([u, 1], F32, tag="negmx")
            nc.scalar.activation(negmx[:], rmx[:], AF.Copy, scale=-0.125)
            p_sb = self.spool.tile([u, LK], F16, tag="psb")
            rss = []
            for (src, h0, nn) in srcs:
                rs = self.kpool.tile([u, 1], F32, tag="rs")
                nc.scalar.activation(p_sb[:, h0:h0 + nn], src[:], AF.Exp,
                                     bias=negmx[:], scale=0.125, accum_out=rs[:])
                rss.append(rs)
            rsum = rss[0]
            if len(rss) == 2:
                r2 = self.kpool.tile([u, 1], F32, tag="rs")
                nc.vector.tensor_tensor(r2[:], rss[0][:], rss[1][:], op=AL.add)
                rsum = r2
            rinv = self.kpool.tile([u, 1], F32, tag="rinv")
            nc.vector.reciprocal(rinv[:], rsum[:])
            # ---- attn^T then upd = P^T' V ----
            pt_sb = self.apool.tile([PTK, NK, u], F16, tag="ptb")
            for ltk in range(NK):
                tp = self.psum_s.tile([PTK, u], F16, tag="psS")
                nc.tensor.matmul(tp[:], p_sb[0:u, ltk * PTK:(ltk + 1) * PTK],
                                 self.ident[0:u, 0:u], is_transpose=True,
                                 start=True, stop=True)
                nc.scalar.copy(pt_sb[:, ltk, :], tp[:])
            ups = self.psum_m.tile([u, DH], F32, tag="pm")
            for ltk in range(NK):
                nc.tensor.matmul(ups[:], pt_sb[:, ltk, :], V[:, ltk, h * DH:(h + 1) * DH],
                                 start=(ltk == 0), stop=(ltk == NK - 1))
            upd = self.kpool.tile([u, DH], F16, tag="upd")
            nc.scalar.activation(upd[:], ups[:], AF.Copy, scale=rinv[:])
            # ---- context + scatter + transpose into OT ----
            if not masked:
                vms = self.psum_m.tile([1, DH], F32, tag="pm")
                for ltk in range(NK):
                    nc.tensor.matmul(vms[:], self.ones[0:PTK, 0:1], V[:, ltk, h * DH:(h + 1) * DH],
                                     start=(ltk == 0), stop=(ltk == NK - 1))
                vm = self.kpool.tile([1, DH], F16, tag="vm")
                nc.scalar.activation(vm[:], vms[:], AF.Copy, scale=1.0 / LK)
            for lt in range(NQ):
                bps = self.psum_m.tile([PTQ, DH], F32, tag="pm")
                if masked:
                    for ltp in range(lt + 1):
                        nc.tensor.matmul(bps[:], ut[:, ltp, lt * PTQ:(lt + 1) * PTQ],
                                         V[:, ltp, h * DH:(h + 1) * DH],
                                         start=(ltp == 0), stop=(ltp == lt))
                else:
                    nc.tensor.matmul(bps[:], self.ones[0:1, 0:PTQ], vm[:], start=True, stop=True)
                ob = self.kpool.tile([PTQ, DH], F32, tag="ob")
                nc.vector.tensor_scalar(ob[:], bps[:], ntop[:, lt:lt + 1], None, op0=AL.mult)
                stp = self.psum_s.tile([u, PTQ], F16, tag="psS")
                nc.tensor.matmul(stp[:], sel[:, lt, :], self.ident[0:PTQ, 0:PTQ],
                                 is_transpose=True, start=True, stop=True)
                selt = self.kpool.tile([u, PTQ], F16, tag="selt")
                nc.scalar.copy(selt[:], stp[:])
                ops_ = self.psum_m.tile([PTQ, DH], F32, tag="pm")
                nc.tensor.matmul(ops_[:], selt[:], upd[:], start=True, stop=True)
                olt = self.kpool.tile([PTQ, DH], F16, tag="olt")
                nc.vector.tensor_tensor(olt[:], ops_[:], ob[:], op=AL.add)
                otp = self.psum_s.tile([DH, PTQ], F16, tag="psS")
                nc.tensor.matmul(otp[:], olt[:], self.ident[0:PTQ, 0:PTQ],
                                 is_transpose=True, start=True, stop=True)
                nc.scalar.copy(OT[r0:r0 + DH, ti, lt * PTQ:(lt + 1) * PTQ], otp[:])
        return OT

    def attn_out_proj(self, aname, OT, Xres, LQ):
        """Wo projection + bias + residual add."""
        nc = self.nc
        bo = self.ld(aname + 'bo', [128, 4], F32, pool=self.cpool)

        def evict(ps, m, h0, nn, out):
            nc.vector.scalar_tensor_tensor(out[:, m, h0:h0 + nn], ps[:], bo[:, m:m + 1],
                                           Xres[:, m, h0:h0 + nn], op0=AL.add, op1=AL.add)
        out = self.rpool.tile([128, 4, LQ], F16, tag="resid")
        self.lin_dmaj(OT, aname + 'wo', LQ, evict=evict, out=out)
        return out


def _transpose_mm(nc, kb, out_ps, in_ap, ident):
    nc.tensor.matmul(out_ps, in_ap, ident, is_transpose=True, start=True, stop=True)


def build_program():
    nc = bacc.Bacc("TRN2", target_bir_lowering=False, debug=False, num_devices=8)
    consts = _host_constants()

    dram = {}

    def din(name, shape, dt=F32):
        dram[name] = nc.dram_tensor(name, list(shape), dt, kind="ExternalInput")

    # constants
    din('iota', [128, 720]); din('ident', [128, 128], F16)
    din('ident32', [128, 128]); din('ones', [128, 128], F16)
    din('ones32', [1, 128])
    for a, sp in ATTN_SPECS.items():
        PT = _pt(sp['LQ'])
        NQ = sp['LQ'] // PT
        din(f'cm_{a}', [PT, NQ, sp['LK']], F16)
        din(f'nm_{a}', [PT, NQ, sp['LK']], F16)
        din(f'lt_{a}', [PT, NQ, sp['LQ']], F16)
        if sp['masked']:
            din(f'ut_{a}', [PT, NQ, sp['LQ']], F16)
            din(f'ic_{a}', [PT, NQ, 1], F16)
    din('pos_e', [128, 4, SEQ], F16); din('pos_d', [128, 4, DEC])
    din('cwd2l', [128, 3, 3, D], F16); din('xdtl', [128, 3, DEC], F16)
    # weights
    for pref in ('e0', 'e1', 'ds', 'dc'):
        for nm in ('wq', 'wk', 'wv', 'wo'):
            din(pref + nm, [128, 4, D], F16)
        for nm in ('bq', 'bk', 'bo'):
            din(pref + nm, [128, 4])
        din(pref + 'bvr', [1, D], F16)
    din('dswql', [128, 4, D], F16)
    din('dswkl', [128, 4, D], F16)
    for pref in ('e0', 'e1', 'd'):
        din(pref + 'w1', [128, 4, DFF], F16)
        din(pref + 'w2', [128, 16, D], F16)
        din(pref + 'b1', [128, 16])
        din(pref + 'b2', [128, 4])
    for pref in ('e0n1', 'e0n2', 'e1n1', 'e1n2', 'encn', 'dn1', 'dn2', 'dn3', 'decn'):
        din(pref + 'g', [128, 4])
        din(pref + 'b', [128, 4])
    din('cwe', [128, 3, 3, D], F16); din('mwe', [MARK, D], F16)
    din('cwd2', [128, 3, 3, D], F16); din('mwd', [MARK, D], F16)
    din('cwd', [128, 4, 3, D], F16); din('bd', [128, 4])
    din('pw', [128, 4, CINP], F16); din('pb', [128, 3])
    # per-core inputs
    din('xet', [128, 3, SEQ])            # x_enc^T padded to 384 channels, f32
    din('xmet', [MARK, SEQ], F16)
    din('xdt', [128, 3, DEC], F16)       # x_dec^T padded, fp16
    din('xmdt', [MARK, DEC], F16)
    out_d = nc.dram_tensor('out', [128, 3, PRED], F32, kind="ExternalOutput")

    with tile.TileContext(nc) as tc:
        with ExitStack() as ctx:
            ctx.enter_context(nc.allow_low_precision(reason="deliberate fp16 kernel"))
            kb = KB(nc, tc, ctx, dram)
            stop = STOP_AFTER

            # ---- RevIN ----
            meanc = kb.cpool.tile([128, 3], F32, tag="meanc")
            sdc = kb.cpool.tile([128, 3], F32, tag="sdc")
            xn = kb.apool.tile([128, 3, SEQ], F16, tag="xnorm")
            for c in range(3):
                xch = kb.spool.tile([128, SEQ], F32, tag="xetc")
                nc.sync.dma_start(out=xch[:], in_=dram['xet'].ap()[:, c, :])
                sm = kb.kpool.tile([128, 1], F32, tag="mx")
                nc.vector.tensor_reduce(sm[:], xch[:], axis=AX.X, op=AL.add)
                nc.scalar.activation(meanc[:, c:c + 1], sm[:], AF.Copy, scale=1.0 / SEQ)
                xc = kb.spool.tile([128, SEQ], F32, tag="xetc")
                nc.vector.tensor_scalar(xc[:], xch[:], meanc[:, c:c + 1], None, op0=AL.subtract)
                var = kb.kpool.tile([128, 1], F32, tag="sm")
                scr2 = kb.spool.tile([128, SEQ], F32, tag="xetc")
                nc.vector.scalar_tensor_tensor(scr2[:], xc[:], 1.0 / SEQ, xc[:],
                                               op0=AL.mult, op1=AL.mult, accum_out=var[:])
                nc.scalar.activation(sdc[:, c:c + 1], var[:], AF.Sqrt, bias=kb.epsc[:], scale=1.0)
                rsd = kb.kpool.tile([128, 1], F32, tag="cgt")
                nc.vector.reciprocal(rsd[:], sdc[:, c:c + 1])
                nc.vector.tensor_scalar(xn[:, c, :], xc[:], rsd[:], None, op0=AL.mult)

            # ---- encoder embed ----
            xmet = kb.ld('xmet', [MARK, SEQ], F16, pool=kb.cpool)
            X, _ = kb.embed(xn, xmet, 'cwe', 'mwe', 'pos_e', SEQ, 3)
            kb.dump_dmaj('X0', X, 4, SEQ)
            if stop != 'embed':
                # ---- encoder layer 0 ----
                OT = kb.attention('e0', X, X, SEQ, SEQ)
                R = kb.attn_out_proj('e0', OT, X, SEQ)
                X = kb.layernorm(R, 'e0n1g', 'e0n1b', SEQ, "resid")
                R = kb.ffn(X, 'e0', SEQ)
                X = kb.layernorm(R, 'e0n2g', 'e0n2b', SEQ, "resid")
                kb.dump_dmaj('X_enc0', X, 4, SEQ)
            if stop not in ('embed', 'enc0'):
                # ---- distill conv ----
                bd = kb.ld('bd', [128, 4], F32, pool=kb.cpool)
                xpad = kb.apool.tile([128, 4, SEQ + 2], F16, tag="xa")
                for c in range(4):
                    nc.vector.tensor_copy(xpad[:, c, 1:SEQ + 1], X[:, c, :])
                    nc.vector.tensor_copy(xpad[:, c, 0:1], X[:, c, SEQ - 1:SEQ])
                    nc.vector.tensor_copy(xpad[:, c, SEQ + 1:SEQ + 2], X[:, c, 0:1])
                cw = kb.bigp.tile([128, 4, 3, D], F16, tag="big")
                nc.sync.dma_start(out=cw[:], in_=dram['cwd'].ap())
                X2 = kb.rpool.tile([128, 4, 360], F16, tag="resid")
                for m in range(4):
                    celu = kb.spool.tile([128, SEQ], F16, tag="celu")
                    for (h0, nn) in _halves(SEQ):
                        ps = kb.psum_n.tile([128, nn], F32, tag="pn")
                        first = True
                        for k in range(3):
                            for c in range(4):
                                nc.tensor.matmul(ps[:], cw[:, c, k, m * 128:(m + 1) * 128],
                                                 xpad[:, c, k + h0:k + h0 + nn],
                                                 start=first, stop=(k == 2 and c == 3))
                                first = False
                        nc.scalar.activation(celu[:, h0:h0 + nn], ps[:], AF.Identity,
                                             bias=bd[:, m:m + 1], scale=1.0)
                    # ELU
                    t1 = kb.spool.tile([128, SEQ], F16, tag="scr")
                    nc.vector.tensor_scalar(t1[:], celu[:], 0.0, None, op0=AL.min)
                    t2 = kb.spool.tile([128, SEQ], F16, tag="scr")
                    nc.scalar.activation(t2[:], t1[:], AF.Exp)
                    t3 = kb.spool.tile([128, SEQ], F16, tag="celu2")
                    nc.scalar.activation(t3[:], celu[:], AF.Relu)
                    el = kb.spool.tile([128, SEQ], F16, tag="celu")
                    nc.vector.scalar_tensor_tensor(el[:], t3[:], -1.0, t2[:], op0=AL.add, op1=AL.add)
                    # maxpool k=3 s=2 pad=1
                    e3 = el[:].rearrange("p (n two) -> p n two", two=2)
                    nc.vector.tensor_tensor(X2[:, m, :], e3[:, :, 0], e3[:, :, 1], op=AL.max)
                    nc.vector.tensor_tensor(X2[:, m, 1:360], X2[:, m, 1:360],
                                            el[:, 1:719].rearrange("p (n two) -> p n two", two=2)[:, :, 0],
                                            op=AL.max)
                X = X2
                kb.dump_dmaj('X_dist', X, 4, 360)
            if stop not in ('embed', 'enc0', 'distill'):
                # ---- encoder layer 1 ----
                OT = kb.attention('e1', X, X, 360, 360)
                R = kb.attn_out_proj('e1', OT, X, 360)
                X = kb.layernorm(R, 'e1n1g', 'e1n1b', 360, "resid")
                R = kb.ffn(X, 'e1', 360)
                X = kb.layernorm(R, 'e1n2g', 'e1n2b', 360, "resid")
                cross = kb.layernorm(X, 'encng', 'encnb', 360, "cross")
                kb.dump_dmaj('cross', cross, 4, 360)
            if stop in ('dembed', 'ds', 'dc', 'all'):
                # ---- decoder embed ----
                xdt = kb.ld('xdt', [128, 3, DEC], F16, pool=kb.apool, tag="xnorm")
                xdtl = kb.ld('xdtl', [128, 3, DEC], F16, pool=kb.apool, tag="xnorml")
                xmdt = kb.ld('xmdt', [MARK, DEC], F16, pool=kb.cpool)
                Y, Ylo = kb.embed(xdt, xmdt, 'cwd2', 'mwd', 'pos_d', DEC, 3,
                                  xtl=xdtl, want_lo=True)
                kb.dump_dmaj('Y0', Y, 4, DEC)
            if stop in ('ds', 'dc', 'all'):
                OT = kb.attention('ds', Y, Y, DEC, DEC, XQlo=Ylo, XKVlo=Ylo)
                R = kb.attn_out_proj('ds', OT, Y, DEC)
                Y = kb.layernorm(R, 'dn1g', 'dn1b', DEC, "resid")
                kb.dump_dmaj('Y_ds', Y, 4, DEC)
            if stop in ('dc', 'all'):
                OT = kb.attention('dc', Y, cross, DEC, 360)
                R = kb.attn_out_proj('dc', OT, Y, DEC)
                Y = kb.layernorm(R, 'dn2g', 'dn2b', DEC, "resid")
                R = kb.ffn(Y, 'd', DEC)
                Y = kb.layernorm(R, 'dn3g', 'dn3b', DEC, "resid")
                Y = kb.layernorm(Y, 'decng', 'decnb', DEC, "resid")
                kb.dump_dmaj('Y_out', Y, 4, DEC)
            if stop == 'all':
                # ---- projection + de-norm, first 336 cols only ----
                pb = kb.ld('pb', [128, 3], F32, pool=kb.cpool)
                pwd = dram['pw']
                outsb = kb.spool.tile([128, 3, PRED], F32, tag="outsb", bufs=1)
                wt = kb.wpool.tile([128, 4, CINP], F16, tag="wstream")
                nc.sync.dma_start(out=wt[:], in_=pwd.ap())
                for m in range(3):
                    ps = kb.psum_n.tile([128, PRED], F32, tag="pn")
                    for kc in range(4):
                        nc.tensor.matmul(ps[:], wt[:, kc, m * 128:(m + 1) * 128],
                                         Y[:, kc, 0:PRED], start=(kc == 0), stop=(kc == 3))
                    t = kb.spool.tile([128, PRED], F32, tag="psb")
                    nc.vector.tensor_scalar(t[:], ps[:], pb[:, m:m + 1], None, op0=AL.add)
                    nc.vector.tensor_scalar(outsb[:, m, :], t[:], sdc[:, m:m + 1],
                                            meanc[:, m:m + 1], op0=AL.mult, op1=AL.add)
                    nc.sync.dma_start(out=out_d.ap()[:, m, :], in_=outsb[:, m, :])

    nc.compile()
    return nc


_PROG = None


def _in_maps(x_enc, x_mark_enc, x_dec, x_mark_dec, params):
    consts = _host_constants()
    w = _pack_params(params)
    x_enc = np.asarray(x_enc, np.float32)
    x_me = np.asarray(x_mark_enc, np.float32)
    x_dec = np.asarray(x_dec, np.float32)
    x_md = np.asarray(x_mark_dec, np.float32)
    base = dict(consts)
    base.update(w)
    in_maps = []
    for b in range(B):
        m = dict(base)
        xe = np.zeros((CINP, SEQ), np.float32)
        xe[:CIN] = x_enc[b].T
        m['xet'] = np.ascontiguousarray(xe.reshape(3, 128, SEQ).transpose(1, 0, 2))
        m['xmet'] = _f16(x_me[b].T)
        xd = np.zeros((CINP, DEC), np.float32)
        xd[:CIN] = x_dec[b].T
        xd = np.ascontiguousarray(xd.reshape(3, 128, DEC).transpose(1, 0, 2))
        xdh = xd.astype(np.float16)
        m['xdt'] = xdh
        m['xdtl'] = (xd - xdh.astype(np.float32)).astype(np.float16)
        m['xmdt'] = _f16(x_md[b].T)
        in_maps.append(m)
    return in_maps


def _unpack_out(results):
    outs = []
    for b in range(B):
        o = results[b]['out']              # [128, 3, 336]
        o = o.transpose(1, 0, 2).reshape(CINP, PRED)[:COUT]   # [321, 336]
        outs.append(o.T)                    # [336, 321]
    return np.stack(outs, 0).astype(np.float32)


def get_program():
    global _PROG
    if _PROG is None:
        _PROG = build_program()
    return _PROG


def kernel(x_enc, x_mark_enc, x_dec, x_mark_dec, params):
    nc = get_program()
    in_maps = _in_maps(x_enc, x_mark_enc, x_dec, x_mark_dec, params)
    res = bass_utils.run_bass_kernel_spmd(nc, in_maps, core_ids=list(range(B)))
    return _unpack_out(res.results)


if __name__ == '__main__':
    pass


# revision 41
# speedup vs baseline: 1.0536x; 1.0536x over previous
"""Trainium2 Bass kernel for nn_Model_26070451487349 (Informer, sparse_attention).

Strategy: data-parallel over batch (B=8) across 8 NeuronCores; each core runs
the full Informer forward for one batch element. Activations live in SBUF
"d-major" layout [D partitions, L free]. ProbSparse attention top-k/gather/
scatter are done with full-score matmuls + host-precomputed constant masks
(sample-count matrix, sampled-position mask, triangular matrices); the top-k
set is computed with exact-fp32 rank counting (count of strictly-greater M
values + index tie-break), matching jax.lax.top_k semantics.

Matmul operands are fp16 (PE runs 1 cycle/col at any N; ~11-bit mantissa,
same class as f32r but half the memory); PSUM accumulation is fp32 and all
top-k comparisons are exact fp32.
"""
import sys
import os
import numpy as np
from contextlib import ExitStack

for _p in ("/opt/trn_rl_repo",):
    if _p not in sys.path:
        sys.path.insert(0, _p)

import concourse.bass as bass
import concourse.tile as tile
from concourse import bass_utils, bacc, mybir

F32 = mybir.dt.float32
F16 = mybir.dt.float16
AL = mybir.AluOpType
AF = mybir.ActivationFunctionType
AX = mybir.AxisListType

# ---- model dims (hardcoded from reference) ----
B = 8
SEQ = 720          # encoder length
DEC = 672          # decoder length
PRED = 336
CIN = 321
CINP = 384         # padded to 3x128
COUT = 321
MARK = 4
D = 512
H = 8
DH = 64
DFF = 2048
FACTOR = 5
EPS = 1e-5
NEG = -60000.0     # "-inf" for fp16 sampled-mask

# attention specs: (LQ, LK, u, Upart, fold, masked)
ATTN_SPECS = {
    'e0': dict(LQ=720, LK=720, u=35, Up=35, fold=0, masked=False),
    'e1': dict(LQ=360, LK=360, u=30, Up=30, fold=1, masked=False),
    'ds': dict(LQ=672, LK=672, u=35, Up=35, fold=100, masked=True),
    'dc': dict(LQ=672, LK=360, u=35, Up=30, fold=200, masked=False),
}


def _pt(L):
    # partition-tile size for a sequence length
    return 120 if L % 120 == 0 else 112


def _halves(L):
    if L <= 512:
        return [(0, L)]
    h = L // 2
    return [(0, h), (h, L - h)]


# ---------------------------------------------------------------------------
# host-side constants
# ---------------------------------------------------------------------------

def _pos_embedding_np(L, d):
    pos = np.arange(L, dtype=np.float32)[:, None]
    div = np.exp(np.arange(0, d, 2, dtype=np.float32) * -(np.log(10000.0) / d))
    pe = np.zeros((L, d), dtype=np.float32)
    pe[:, 0::2] = np.sin(pos * div)
    pe[:, 1::2] = np.cos(pos * div)
    return pe


def _tile_lm(a, PT):
    # [L, K] -> [PT, L//PT, K]  (partition-tiled rows)
    L, Kd = a.shape
    n = L // PT
    return np.ascontiguousarray(a.reshape(n, PT, Kd).transpose(1, 0, 2))


def _dmaj(a):
    # [Din, Dout] -> [128, Din//128, Dout]
    Din, Dout = a.shape
    return np.ascontiguousarray(a.reshape(Din // 128, 128, Dout).transpose(1, 0, 2))


_CONSTS = None


def _host_constants():
    global _CONSTS
    if _CONSTS is not None:
        return _CONSTS
    import jax
    cpu = jax.devices('cpu')[0]
    c = {}
    iota = np.tile(np.arange(768, dtype=np.float32), (128, 1))
    c['iota'] = iota[:, :720].copy()
    c['ident'] = np.eye(128, dtype=np.float16)
    c['ident32'] = np.eye(128, dtype=np.float32)
    c['ones'] = np.ones((128, 128), dtype=np.float16)
    c['ones32'] = np.ones((1, 128), dtype=np.float32)
    with jax.default_device(cpu):
        k0 = jax.random.key(42)
        for name, sp in ATTN_SPECS.items():
            LQ, LK, u, Up = sp['LQ'], sp['LK'], sp['u'], sp['Up']
            key = jax.random.fold_in(k0, sp['fold'])
            idx = np.array(jax.random.randint(key, (LQ, Up), 0, LK))
            cm = np.zeros((LQ, LK), np.float32)
            np.add.at(cm, (np.arange(LQ)[:, None], idx), 1.0)
            nm = np.where(cm > 0, 0.0, NEG).astype(np.float32)
            PT = _pt(LQ)
            c[f'cm_{name}'] = _tile_lm(cm, PT).astype(np.float16)
            c[f'nm_{name}'] = _tile_lm(nm, PT).astype(np.float16)
            lt = np.tril(np.ones((LQ, LQ), np.float32), -1)
            c[f'lt_{name}'] = _tile_lm(lt, PT).astype(np.float16)
            if sp['masked']:
                ut = (np.arange(LQ)[:, None] <= np.arange(LQ)[None, :]).astype(np.float32)
                c[f'ut_{name}'] = _tile_lm(ut, PT).astype(np.float16)
                ic = np.arange(LQ, dtype=np.float32).reshape(LQ, 1)
                c[f'ic_{name}'] = _tile_lm(ic, PT).astype(np.float16)
    pe = _pos_embedding_np(SEQ, D)     # [720, 512]
    c['pos_e'] = _dmaj(pe.T.copy().astype(np.float32)).astype(np.float16)
    pd = _pos_embedding_np(DEC, D)
    c['pos_d'] = _dmaj(pd.T.copy().astype(np.float32))
    _CONSTS = c
    return c


def _f16(a):
    return np.ascontiguousarray(a).astype(np.float16)


def _bcol(b, Mc):
    # bias [Dout] -> [128, Mc] (column m holds bias of m-th 128-chunk), fp32
    bp = np.zeros(Mc * 128, np.float32)
    bp[:b.shape[0]] = b
    return np.ascontiguousarray(bp.reshape(Mc, 128).T)


def _pack_params(params):
    p = {k: {kk: np.asarray(vv, np.float32) if not isinstance(vv, (dict, list)) else vv
             for kk, vv in v.items()} if isinstance(v, dict) else v
         for k, v in params.items()}

    w = {}

    def attn_w(pref, a):
        for nm in ('wq', 'wk', 'wv', 'wo'):
            w[f'{pref}{nm}'] = _f16(_dmaj(np.asarray(a[nm], np.float32)))
        for nm in ('bq', 'bk', 'bo'):
            w[f'{pref}{nm}'] = _bcol(np.asarray(a[nm], np.float32), 4)
        w[f'{pref}bvr'] = _f16(np.asarray(a['bv'], np.float32).reshape(1, D))

    def ffn_w(pref, a):
        w[f'{pref}w1'] = _f16(_dmaj(np.asarray(a['w1'], np.float32)))
        w[f'{pref}w2'] = _f16(_dmaj(np.asarray(a['w2'], np.float32)))
        w[f'{pref}b1'] = _bcol(np.asarray(a['b1'], np.float32), 16)
        w[f'{pref}b2'] = _bcol(np.asarray(a['b2'], np.float32), 4)

    def ln_w(pref, a):
        w[f'{pref}g'] = _bcol(np.asarray(a['g'], np.float32), 4)
        w[f'{pref}b'] = _bcol(np.asarray(a['b'], np.float32), 4)

    def conv_w(pref, cw, cinp, lo=False):
        # cw [Dout, Cin, 3] -> [128, cinp//128, 3, Dout] fp16 (+ optional lo part)
        cw = np.asarray(cw, np.float32)
        Dout, Cin, _ = cw.shape
        a = np.zeros((3, cinp, Dout), np.float32)
        for k in range(3):
            a[k, :Cin, :] = cw[:, :, k].T
        cinc = cinp // 128
        a = a.reshape(3, cinc, 128, Dout).transpose(2, 1, 0, 3)
        hi = a.astype(np.float16)
        w[pref] = hi
        if lo:
            w[pref + 'l'] = (a - hi.astype(np.float32)).astype(np.float16)

    conv_w('cwe', p['enc_emb']['conv_w'], CINP)
    w['mwe'] = _f16(np.asarray(p['enc_emb']['mark_w'], np.float32))
    conv_w('cwd2', p['dec_emb']['conv_w'], CINP, lo=True)
    w['mwd'] = _f16(np.asarray(p['dec_emb']['mark_w'], np.float32))

    e0, e1 = p['enc_layers'][0], p['enc_layers'][1]
    attn_w('e0', e0['attn']); ffn_w('e0', e0); ln_w('e0n1', e0['norm1']); ln_w('e0n2', e0['norm2'])
    attn_w('e1', e1['attn']); ffn_w('e1', e1); ln_w('e1n1', e1['norm1']); ln_w('e1n2', e1['norm2'])

    cv = p['conv_layers'][0]
    s = np.asarray(cv['bn_g'], np.float32) / np.sqrt(np.float32(1.0) + np.float32(EPS))
    cwd = np.asarray(cv['w'], np.float32) * s[:, None, None]
    conv_w('cwd', cwd, D)
    w['bd'] = _bcol(np.asarray(cv['b'], np.float32) * s + np.asarray(cv['bn_b'], np.float32), 4)

    ln_w('encn', p['enc_norm'])

    dl = p['dec_layers'][0]
    attn_w('ds', dl['self_attn'])
    # hi/lo split for fp32-quality Q/K projections in the masked attention
    for nm in ('wq', 'wk'):
        wf = np.asarray(dl['self_attn'][nm], np.float32)
        hi = wf.astype(np.float16)
        lo = (wf - hi.astype(np.float32)).astype(np.float16)
        w[f'ds{nm}'] = _f16(_dmaj(hi.astype(np.float32)))
        w[f'ds{nm}l'] = _f16(_dmaj(lo.astype(np.float32)))
    attn_w('dc', dl['cross_attn'])
    ffn_w('d', dl)
    ln_w('dn1', dl['norm1']); ln_w('dn2', dl['norm2']); ln_w('dn3', dl['norm3'])
    ln_w('decn', p['dec_norm'])

    pw = np.zeros((D, CINP), np.float32)
    pw[:, :COUT] = np.asarray(p['proj_w'], np.float32)
    w['pw'] = _f16(_dmaj(pw))
    w['pb'] = _bcol(np.asarray(p['proj_b'], np.float32), 3)
    return w


# ---------------------------------------------------------------------------
# kernel builder
# ---------------------------------------------------------------------------

STOP_AFTER = os.environ.get('KSTOP', 'all')   # embed|enc0|distill|enc1|dembed|ds|dc|all
DEBUG = bool(int(os.environ.get('KDEBUG', '0')))


class KB:
    def __init__(self, nc, tc, ctx, dram):
        self.nc = nc
        self.tc = tc
        self.dram = dram
        p = lambda name, bufs: ctx.enter_context(tc.tile_pool(name=name, bufs=bufs))
        self.cpool = p("const", 1)
        self.wpool = p("wstream", 2)
        self.mpool = p("mask", 1)
        self.apool = p("act", 1)      # large per-layer activations (distinct tags)
        self.rpool = p("resid", 2)
        self.spool = p("scr", 2)      # [<=128, <=720] fp32 scratch
        self.kpool = p("small", 3)    # [<=128, <=64] small fp32 tiles
        self.bigp = p("big", 1)       # shared slot: ffn-hidden / conv weights / UT
        self.psum_n = ctx.enter_context(tc.tile_pool(name="psum_n", bufs=4, space="PSUM"))
        self.psum_s = ctx.enter_context(tc.tile_pool(name="psum_s", bufs=2, space="PSUM"))
        self.psum_m = ctx.enter_context(tc.tile_pool(name="psum_m", bufs=2, space="PSUM"))
        self.dbg = {}

        # persistent consts
        self.iota = self.ld('iota', [128, 720], F32, pool=self.cpool)
        self.ident = self.ld('ident', [128, 128], F16, pool=self.cpool)
        self.ident32 = self.ld('ident32', [128, 128], F32, pool=self.cpool)
        self.ones = self.ld('ones', [128, 128], F16, pool=self.cpool)
        self.ones32 = self.ld('ones32', [1, 128], F32, pool=self.cpool)
        self.epsc = self.cpool.tile([128, 1], F32, tag="epsc")
        nc.vector.memset(self.epsc[:], EPS)

    # -- misc helpers --
    def ld(self, name, shape, dt, pool=None, tag=None):
        pool = pool or self.cpool
        t = pool.tile(shape, dt, tag=tag or name)
        ap = self.dram[name].ap()
        self.nc.sync.dma_start(out=t[:], in_=ap)
        return t

    def dump(self, name, ap_or_tile, shape, dt=F32):
        if not DEBUG:
            return
        d = self.nc.dram_tensor(f'dbg_{name}', shape, dt, kind="ExternalOutput")
        self.nc.sync.dma_start(out=d.ap(), in_=ap_or_tile)
        self.dbg[name] = shape

    def dump_dmaj(self, name, X, Mc, L):
        # dump a [128, Mc, L] tile as [Mc*128, L] fp32 dram
        if not DEBUG:
            return
        d = self.nc.dram_tensor(f'dbg_{name}', [Mc * 128, L], F32, kind="ExternalOutput")
        for m in range(Mc):
            s = self.spool.tile([128, L], F32, tag="dmp")
            self.nc.scalar.copy(s[:], X[:, m, :])
            self.nc.sync.dma_start(out=d.ap().rearrange("(mc p) l -> p mc l", p=128)[:, m, :], in_=s[:])

    # -- building blocks --

    def lin_dmaj(self, X, wname, L, Mc=4, Kc=4, evict=None, bname=None, act=AF.Identity,
                 out=None, out_tag=None, odt=F16, wlo=None, xlo=None):
        """Y[., m, l] = act(sum_k W[k, m-chunk].T X[k, l] + b).  X: [128, Kc, L] fp16.
        Streams weights in m-groups of 4 chunks. wlo: optional low-half weight
        tensor (fp32-quality weights via hi+lo fp16 matmuls)."""
        nc = self.nc
        if out is None:
            out = self.apool.tile([128, Mc, L], odt, tag=out_tag or wname[:2] + "_o")
        wd = self.dram[wname]
        bt = None
        if bname is not None:
            bt = self.ld(bname, [128, Mc], F32, pool=self.cpool)
        for g in range((Mc + 3) // 4):
            mg = min(4, Mc - g * 4)
            wt = self.wpool.tile([128, Kc, mg * 128], F16, tag="wstream")
            nc.sync.dma_start(out=wt[:], in_=wd.ap()[:, :, g * 512:g * 512 + mg * 128])
            wt2 = None
            if wlo is not None:
                wt2 = self.wpool.tile([128, Kc, mg * 128], F16, tag="wstream")
                nc.sync.dma_start(out=wt2[:], in_=self.dram[wlo].ap()[:, :, g * 512:g * 512 + mg * 128])
            for mm in range(mg):
                m = g * 4 + mm
                for (h0, nn) in _halves(L):
                    ps = self.psum_n.tile([128, nn], F32, tag="pn")
                    last = (wt2 is None and xlo is None)
                    for kc in range(Kc):
                        nc.tensor.matmul(ps[:], wt[:, kc, mm * 128:(mm + 1) * 128],
                                         X[:, kc, h0:h0 + nn],
                                         start=(kc == 0), stop=(kc == Kc - 1 and last))
                    if xlo is not None:
                        for kc in range(Kc):
                            nc.tensor.matmul(ps[:], wt[:, kc, mm * 128:(mm + 1) * 128],
                                             xlo[:, kc, h0:h0 + nn],
                                             start=False, stop=(kc == Kc - 1 and wt2 is None))
                    if wt2 is not None:
                        for kc in range(Kc):
                            nc.tensor.matmul(ps[:], wt2[:, kc, mm * 128:(mm + 1) * 128],
                                             X[:, kc, h0:h0 + nn],
                                             start=False, stop=(kc == Kc - 1))
                    if evict is not None:
                        evict(ps, m, h0, nn, out)
                    elif bt is not None:
                        nc.scalar.activation(out[:, m, h0:h0 + nn], ps[:], act,
                                             bias=bt[:, m:m + 1], scale=1.0)
                    else:
                        nc.scalar.activation(out[:, m, h0:h0 + nn], ps[:], act)
        return out

    def lin_tokmaj(self, X, wname, brname, L, PT, NT, Kc=4, out_tag=None):
        """V[lt-part, lt, dout] = X^T W + b_row. Returns [PT, NT, 512] fp16."""
        nc = self.nc
        out = self.apool.tile([PT, NT, D], F16, tag=out_tag or "vtok")
        wt = self.wpool.tile([128, Kc, D], F16, tag="wstream")
        nc.sync.dma_start(out=wt[:], in_=self.dram[wname].ap())
        br = self.cpool.tile([1, D], F16, tag=brname)
        nc.sync.dma_start(out=br[:], in_=self.dram[brname].ap())
        for lt in range(NT):
            ps = self.psum_n.tile([PT, D], F32, tag="pn")
            for kc in range(Kc):
                nc.tensor.matmul(ps[:], X[:, kc, lt * PT:(lt + 1) * PT], wt[:, kc, :],
                                 start=(kc == 0), stop=False)
            nc.tensor.matmul(ps[:], self.ones[0:1, 0:PT], br[:], start=False, stop=True)
            nc.scalar.copy(out[:, lt, :], ps[:])
        return out

    def layernorm(self, X, gname, bname, L, out_tag="resid"):
        """post-norm layernorm over d (partition dim), d-major layout."""
        nc = self.nc
        g = self.ld(gname, [128, 4], F32, pool=self.cpool)
        b = self.ld(bname, [128, 4], F32, pool=self.cpool)
        pool = self.rpool if out_tag == "resid" else self.apool
        out = pool.tile([128, 4, L], F16, tag=out_tag)
        for (h0, nn) in _halves(L):
            psr = self.psum_m.tile([1, nn], F32, tag="pm")
            for kc in range(4):
                nc.tensor.matmul(psr[:], self.ones[:, 0:1], X[:, kc, h0:h0 + nn],
                                 start=(kc == 0), stop=(kc == 3))
            mu = self.spool.tile([1, nn], F16, tag="murow")
            nc.scalar.activation(mu[:], psr[:], AF.Copy, scale=1.0 / 512.0)
            # broadcast mean and subtract
            mb = self.psum_m.tile([128, nn], F32, tag="pm")
            nc.tensor.matmul(mb[:], self.ones[0:1, :], mu[:], start=True, stop=True)
            xc = self.apool.tile([128, 4, nn], F16, tag="xa")
            sq = self.spool.tile([128, nn], F16, tag="sqscr")
            psv = self.psum_m.tile([1, nn], F32, tag="pm")
            for kc in range(4):
                nc.vector.tensor_tensor(xc[:, kc, :], X[:, kc, h0:h0 + nn], mb[:], op=AL.subtract)
                nc.vector.tensor_tensor(sq[:], xc[:, kc, :], xc[:, kc, :], op=AL.mult)
                nc.tensor.matmul(psv[:], self.ones[:, 0:1], sq[:], start=(kc == 0), stop=(kc == 3))
                sq = self.spool.tile([128, nn], F16, tag="sqscr")
            sd = self.spool.tile([1, nn], F32, tag="sdrow")
            nc.scalar.activation(sd[:], psv[:], AF.Sqrt, bias=self.epsc[0:1, :], scale=1.0 / 512.0)
            rstd = self.spool.tile([1, nn], F16, tag="rstdrow")
            nc.vector.reciprocal(rstd[:], sd[:])
            rb = self.psum_m.tile([128, nn], F32, tag="pm")
            nc.tensor.matmul(rb[:], self.ones[0:1, :], rstd[:], start=True, stop=True)
            for kc in range(4):
                t = self.spool.tile([128, nn], F32, tag="sqscr")
                nc.vector.tensor_tensor(t[:], xc[:, kc, :], rb[:], op=AL.mult)
                nc.scalar.activation(out[:, kc, h0:h0 + nn], t[:], AF.Identity,
                                     bias=b[:, kc:kc + 1], scale=g[:, kc:kc + 1])
        return out

    def ffn(self, X, pref, L):
        """gelu(X W1 + b1) W2 + b2 + X, returns pre-LN residual sum tile."""
        nc = self.nc
        b1 = self.ld(pref + 'b1', [128, 16], F32, pool=self.cpool)
        b2 = self.ld(pref + 'b2', [128, 4], F32, pool=self.cpool)
        out = self.rpool.tile([128, 4, L], F16, tag="resid")
        w1d = self.dram[pref + 'w1']
        w2d = self.dram[pref + 'w2']
        for (h0, nn) in _halves(L):
            ht = self.bigp.tile([128, 16, nn], F16, tag="big")
            for g in range(4):
                wt = self.wpool.tile([128, 4, 512], F16, tag="wstream")
                nc.sync.dma_start(out=wt[:], in_=w1d.ap()[:, :, g * 512:(g + 1) * 512])
                for mm in range(4):
                    mf = g * 4 + mm
                    ps = self.psum_n.tile([128, nn], F32, tag="pn")
                    for kc in range(4):
                        nc.tensor.matmul(ps[:], wt[:, kc, mm * 128:(mm + 1) * 128],
                                         X[:, kc, h0:h0 + nn], start=(kc == 0), stop=(kc == 3))
                    nc.scalar.activation(ht[:, mf, :], ps[:], AF.Gelu,
                                         bias=b1[:, mf:mf + 1], scale=1.0)
            pss = [self.psum_n.tile([128, nn], F32, tag="pn", name=f"pss{i}") for i in range(4)]
            for g in range(4):
                wt = self.wpool.tile([128, 4, 512], F16, tag="wstream")
                nc.sync.dma_start(out=wt[:], in_=w2d.ap()[:, g * 4:(g + 1) * 4, :])
                for m in range(4):
                    for kk in range(4):
                        nc.tensor.matmul(pss[m][:], wt[:, kk, m * 128:(m + 1) * 128],
                                         ht[:, g * 4 + kk, :],
                                         start=(g == 0 and kk == 0), stop=(g == 3 and kk == 3))
            for m in range(4):
                nc.vector.scalar_tensor_tensor(out[:, m, h0:h0 + nn], pss[m][:],
                                               b2[:, m:m + 1], X[:, m, h0:h0 + nn],
                                               op0=AL.add, op1=AL.add)
        return out

    def embed(self, xt, markt, cwname, mwname, posname, L, Cc, xtl=None, want_lo=False):
        """token conv3 (circular) + pos + mark embedding -> [128, 4, L] fp16.
        With xtl/want_lo: hi/lo input+weights for fp32-quality output; returns
        (hi, lo) fp16 pair whose sum is the fp32-accurate embedding."""
        nc = self.nc
        xpad = self.apool.tile([128, Cc, L + 2], F16, tag="xa")
        for c in range(Cc):
            nc.vector.tensor_copy(xpad[:, c, 1:L + 1], xt[:, c, :])
            nc.vector.tensor_copy(xpad[:, c, 0:1], xt[:, c, L - 1:L])
            nc.vector.tensor_copy(xpad[:, c, L + 1:L + 2], xt[:, c, 0:1])
        xpadl = None
        if xtl is not None:
            xpadl = self.apool.tile([128, Cc, L + 2], F16, tag="xal")
            for c in range(Cc):
                nc.vector.tensor_copy(xpadl[:, c, 1:L + 1], xtl[:, c, :])
                nc.vector.tensor_copy(xpadl[:, c, 0:1], xtl[:, c, L - 1:L])
                nc.vector.tensor_copy(xpadl[:, c, L + 1:L + 2], xtl[:, c, 0:1])
        cw = self.bigp.tile([128, Cc, 3, D], F16, tag="big")
        nc.sync.dma_start(out=cw[:], in_=self.dram[cwname].ap())
        cwl = None
        if xtl is not None:
            cwl = self.apool.tile([128, Cc, 3, D], F16, tag="cwl")
            nc.sync.dma_start(out=cwl[:], in_=self.dram[cwname + 'l'].ap())
        mw = self.cpool.tile([MARK, D], F16, tag=mwname)
        nc.sync.dma_start(out=mw[:], in_=self.dram[mwname].ap())
        pdt = F32 if want_lo else F16
        pos = self.ld(posname, [128, 4, L], pdt, pool=self.cpool, tag="pos")
        out = self.rpool.tile([128, 4, L], F16, tag="resid")
        out_lo = self.apool.tile([128, 4, L], F16, tag="xl", name="out_lo") if want_lo else None
        for m in range(4):
            for (h0, nn) in _halves(L):
                ps = self.psum_n.tile([128, nn], F32, tag="pn")
                first = True
                for k in range(3):
                    for c in range(Cc):
                        nc.tensor.matmul(ps[:], cw[:, c, k, m * 128:(m + 1) * 128],
                                         xpad[:, c, k + h0:k + h0 + nn],
                                         start=first, stop=False)
                        first = False
                        if xtl is not None:
                            nc.tensor.matmul(ps[:], cw[:, c, k, m * 128:(m + 1) * 128],
                                             xpadl[:, c, k + h0:k + h0 + nn],
                                             start=False, stop=False)
                            nc.tensor.matmul(ps[:], cwl[:, c, k, m * 128:(m + 1) * 128],
                                             xpad[:, c, k + h0:k + h0 + nn],
                                             start=False, stop=False)
                nc.tensor.matmul(ps[:], mw[:, m * 128:(m + 1) * 128], markt[:, h0:h0 + nn],
                                 start=False, stop=True)
                if want_lo:
                    t = self.spool.tile([128, nn], F32, tag="scr")
                    nc.vector.tensor_tensor(t[:], ps[:], pos[:, m, h0:h0 + nn], op=AL.add)
                    nc.vector.tensor_copy(out[:, m, h0:h0 + nn], t[:])
                    nc.vector.tensor_tensor(out_lo[:, m, h0:h0 + nn], t[:],
                                            out[:, m, h0:h0 + nn], op=AL.subtract)
                else:
                    nc.vector.tensor_tensor(out[:, m, h0:h0 + nn], ps[:], pos[:, m, h0:h0 + nn],
                                            op=AL.add)
        return out, out_lo

    def attention(self, aname, XQ, XKV, LQ, LK, XQlo=None, XKVlo=None):
        """ProbSparse attention. Returns OT d-major [128, 4, LQ] fp16 (pre-Wo)."""
        nc = self.nc
        sp = ATTN_SPECS[aname]
        u, masked = sp['u'], sp['masked']
        PTQ, NQ = _pt(LQ), LQ // _pt(LQ)
        PTK, NK = _pt(LK), LK // _pt(LK)
        hvK = _halves(LK)

        # high-precision Q/K path for the masked attention: its top-k flips
        # swap attention vs cumsum rows (large damage), so Q/K stay fp32 and
        # Wq/Wk (and the input) are applied as hi+lo fp16 pairs.
        hiprec = masked
        qdt = F32 if hiprec else F16
        QT = self.lin_dmaj(XQ, aname + 'wq', LQ, bname=aname + 'bq', out_tag="qt",
                           odt=qdt, wlo=(aname + 'wql') if hiprec else None,
                           xlo=XQlo if hiprec else None)
        KT = self.lin_dmaj(XKV, aname + 'wk', LK, bname=aname + 'bk', out_tag="kt",
                           odt=qdt, wlo=(aname + 'wkl') if hiprec else None,
                           xlo=XKVlo if hiprec else None)
        V = self.lin_tokmaj(XKV, aname + 'wv', aname + 'bvr', LK, PTK, NK)

        cm = self.mpool.tile([PTQ, NQ, LK], F16, tag="cm")
        nc.sync.dma_start(out=cm[:], in_=self.dram['cm_' + aname].ap())
        nm = self.mpool.tile([PTQ, NQ, LK], F16, tag="nm")
        nc.sync.dma_start(out=nm[:], in_=self.dram['nm_' + aname].ap())
        ltm = self.mpool.tile([PTQ, NQ, LQ], F16, tag="ltm")
        nc.sync.dma_start(out=ltm[:], in_=self.dram['lt_' + aname].ap())
        if masked:
            ut = self.bigp.tile([PTQ, NQ, LQ], F16, tag="big")
            nc.sync.dma_start(out=ut[:], in_=self.dram['ut_' + aname].ap())
            ic = self.cpool.tile([PTQ, NQ, 1], F16, tag="ic_" + aname)
            nc.sync.dma_start(out=ic[:], in_=self.dram['ic_' + aname].ap())

        OT = self.apool.tile([128, 4, LQ], F16, tag="ot")

        for h in range(H):
            ti, r0 = (h * DH) // 128, (h * DH) % 128
            # ---- S = q k^T (token-major [lq, lk]); M stats read fp32 psum ----
            s_sb = self.apool.tile([PTQ, NQ, LK], F16, tag="s_sb", bufs=2)
            mcol = self.spool.tile([PTQ, NQ], F32, tag="mcol")
            for lt in range(NQ):
                mxs, sms = [], []
                for (h0, nn) in hvK:
                    ps = self.psum_s.tile([PTQ, nn], F32, tag="psS")
                    nc.tensor.matmul(ps[:], QT[r0:r0 + DH, ti, lt * PTQ:(lt + 1) * PTQ],
                                     KT[r0:r0 + DH, ti, h0:h0 + nn], start=True, stop=True)
                    nc.scalar.copy(s_sb[:, lt, h0:h0 + nn], ps[:])
                    mx = self.kpool.tile([PTQ, 1], F32, tag="mx", name=f"mx{lt}")
                    sm = self.kpool.tile([PTQ, 1], F32, tag="sm", name=f"sm{lt}")
                    if hiprec:
                        scr = self.spool.tile([PTQ, nn], F32, tag="scr")
                        nc.vector.tensor_tensor(scr[:], ps[:], nm[:, lt, h0:h0 + nn], op=AL.add)
                        nc.vector.tensor_reduce(mx[:], scr[:], axis=AX.X, op=AL.max)
                        scr2 = self.spool.tile([PTQ, nn], F32, tag="scr")
                        nc.vector.scalar_tensor_tensor(scr2[:], ps[:], 1.0 / LK, cm[:, lt, h0:h0 + nn],
                                                       op0=AL.mult, op1=AL.mult, accum_out=sm[:])
                    else:
                        scr = self.spool.tile([PTQ, nn], F16, tag="scr")
                        nc.vector.tensor_tensor(scr[:], s_sb[:, lt, h0:h0 + nn],
                                                nm[:, lt, h0:h0 + nn], op=AL.add)
                        nc.vector.tensor_reduce(mx[:], scr[:], axis=AX.X, op=AL.max)
                        scr2 = self.spool.tile([PTQ, nn], F16, tag="scr")
                        nc.vector.scalar_tensor_tensor(scr2[:], s_sb[:, lt, h0:h0 + nn], 1.0 / LK,
                                                       cm[:, lt, h0:h0 + nn],
                                                       op0=AL.mult, op1=AL.mult, accum_out=sm[:])
                    mxs.append(mx)
                    sms.append(sm)
                if len(hvK) == 2:
                    nc.vector.tensor_tensor(mxs[0][:], mxs[0][:], mxs[1][:], op=AL.max)
                    nc.vector.tensor_tensor(sms[0][:], sms[0][:], sms[1][:], op=AL.add)
                nc.vector.tensor_tensor(mcol[:, lt:lt + 1], mxs[0][:], sms[0][:], op=AL.subtract)
            # ---- exact top-u rank ----
            mrow = self.spool.tile([1, LQ], F32, tag="mrow", bufs=1)
            for lt in range(NQ):
                tp = self.psum_m.tile([1, PTQ], F32, tag="pm")
                nc.tensor.matmul(tp[:], mcol[:, lt:lt + 1], self.ident32[0:PTQ, 0:PTQ],
                                 is_transpose=True, start=True, stop=True)
                nc.scalar.copy(mrow[0:1, lt * PTQ:(lt + 1) * PTQ], tp[:])
            hvQ = _halves(LQ)
            mbs = []
            for qi, (q0, qn) in enumerate(hvQ):
                mbp = self.psum_s.tile([PTQ, qn], F32, tag="psS", name=f"mbp{qi}")
                nc.tensor.matmul(mbp[:], self.ones32[0:1, 0:PTQ], mrow[0:1, q0:q0 + qn],
                                 start=True, stop=True)
                mbs.append((mbp, q0, qn))
            sel = self.apool.tile([PTQ, NQ, u], F16, tag="sel")
            ntop = self.spool.tile([PTQ, NQ], F32, tag="ntop")
            for lt in range(NQ):
                rank = self.kpool.tile([PTQ, 1], F32, tag="rank")
                for qi, (mbp, q0, qn) in enumerate(mbs):
                    scr = self.spool.tile([PTQ, qn], F16, tag="scr")
                    cgt = self.kpool.tile([PTQ, 1], F32, tag="cgt", name=f"cgt{qi}")
                    nc.vector.tensor_scalar(scr[:], mbp[:], mcol[:, lt:lt + 1], 0.0,
                                            op0=AL.is_gt, op1=AL.add, accum_out=cgt[:])
                    scr2 = self.spool.tile([PTQ, qn], F16, tag="scr")
                    ceq = self.kpool.tile([PTQ, 1], F32, tag="ceq", name=f"ceq{qi}")
                    nc.vector.scalar_tensor_tensor(scr2[:], mbp[:], mcol[:, lt:lt + 1],
                                                   ltm[:, lt, q0:q0 + qn],
                                                   op0=AL.is_equal, op1=AL.mult, accum_out=ceq[:])
                    if qi == 0:
                        nc.vector.tensor_tensor(rank[:], cgt[:], ceq[:], op=AL.add)
                    else:
                        nc.vector.tensor_tensor(rank[:], rank[:], cgt[:], op=AL.add)
                        nc.vector.tensor_tensor(rank[:], rank[:], ceq[:], op=AL.add)
                nc.vector.tensor_scalar(sel[:, lt, :], self.iota[0:PTQ, 0:u], rank[:], None,
                                        op0=AL.is_equal)
                nc.vector.tensor_scalar(ntop[:, lt:lt + 1], rank[:], float(u) - 0.5, None,
                                        op0=AL.is_ge)
            if DEBUG:
                self.dump(f'{aname}_mcol_h{h}', mcol[:], [PTQ, NQ])
            # ---- scores = Sel^T S  (+ causal mask) -> softmax ----
            if masked:
                tps = self.psum_m.tile([u, 1], F32, tag="pm")
                for lt in range(NQ):
                    nc.tensor.matmul(tps[:], sel[:, lt, :], ic[:, lt, :],
                                     start=(lt == 0), stop=(lt == NQ - 1))
                tcol = self.kpool.tile([u, 1], F32, tag="tcol")
                nc.scalar.copy(tcol[:], tps[:])
            srcs = []
            for (h0, nn) in hvK:
                ps = self.psum_s.tile([u, nn], F32, tag="psS")
                for lt in range(NQ):
                    nc.tensor.matmul(ps[:], sel[:, lt, :], s_sb[:, lt, h0:h0 + nn],
                                     start=(lt == 0), stop=(lt == NQ - 1))
                if masked:
                    madd = self.spool.tile([u, nn], F32, tag="madd")
                    nc.vector.tensor_scalar(madd[:], self.iota[0:u, h0:h0 + nn], tcol[:], -1e30,
                                            op0=AL.is_gt, op1=AL.mult)
                    sc = self.spool.tile([u, nn], F32, tag="scmask")
                    nc.vector.tensor_tensor(sc[:], ps[:], madd[:], op=AL.add)
                    srcs.append((sc, h0, nn))
                else:
                    srcs.append((ps, h0, nn))
            rmxs = []
            for (src, h0, nn) in srcs:
                r = self.kpool.tile([u, 1], F32, tag="rmx")
                nc.vector.tensor_reduce(r[:], src[:], axis=AX.X, op=AL.max)
                rmxs.append(r)
            rmx = rmxs[0]
            if len(rmxs) == 2:
                rmx2 = self.kpool.tile([u, 1], F32, tag="rmx")
                nc.vector.tensor_tensor(rmx2[:], rmxs[0][:], rmxs[1][:], op=AL.max)
                rmx = rmx2
            negmx = self.kpool.tile([u, 1], F32, tag="negmx")
            nc.scalar.activation(negmx[:], rmx[:], AF.Copy, scale=-0.125)
            p_sb = self.spool.tile([u, LK], F16, tag="psb")
            rss = []
            for (src, h0, nn) in srcs:
                rs = self.kpool.tile([u, 1], F32, tag="rs")
                nc.scalar.activation(p_sb[:, h0:h0 + nn], src[:], AF.Exp,
                                     bias=negmx[:], scale=0.125, accum_out=rs[:])
                rss.append(rs)
            rsum = rss[0]
            if len(rss) == 2:
                r2 = self.kpool.tile([u, 1], F32, tag="rs")
                nc.vector.tensor_tensor(r2[:], rss[0][:], rss[1][:], op=AL.add)
                rsum = r2
            rinv = self.kpool.tile([u, 1], F32, tag="rinv")
            nc.vector.reciprocal(rinv[:], rsum[:])
            # ---- attn^T then upd = P^T' V ----
            pt_sb = self.apool.tile([PTK, NK, u], F16, tag="ptb")
            for ltk in range(NK):
                tp = self.psum_s.tile([PTK, u], F16, tag="psS")
                nc.tensor.matmul(tp[:], p_sb[0:u, ltk * PTK:(ltk + 1) * PTK],
                                 self.ident[0:u, 0:u], is_transpose=True,
                                 start=True, stop=True)
                nc.scalar.copy(pt_sb[:, ltk, :], tp[:])
            ups = self.psum_m.tile([u, DH], F32, tag="pm")
            for ltk in range(NK):
                nc.tensor.matmul(ups[:], pt_sb[:, ltk, :], V[:, ltk, h * DH:(h + 1) * DH],
                                 start=(ltk == 0), stop=(ltk == NK - 1))
            upd = self.kpool.tile([u, DH], F16, tag="upd")
            nc.scalar.activation(upd[:], ups[:], AF.Copy, scale=rinv[:])
            # ---- context + scatter + transpose into OT ----
            if not masked:
                vms = self.psum_m.tile([1, DH], F32, tag="pm")
                for ltk in range(NK):
                    nc.tensor.matmul(vms[:], self.ones[0:PTK, 0:1], V[:, ltk, h * DH:(h + 1) * DH],
                                     start=(ltk == 0), stop=(ltk == NK - 1))
                vm = self.kpool.tile([1, DH], F16, tag="vm")
                nc.scalar.activation(vm[:], vms[:], AF.Copy, scale=1.0 / LK)
            for lt in range(NQ):
                bps = self.psum_m.tile([PTQ, DH], F32, tag="pm")
                if masked:
                    for ltp in range(lt + 1):
                        nc.tensor.matmul(bps[:], ut[:, ltp, lt * PTQ:(lt + 1) * PTQ],
                                         V[:, ltp, h * DH:(h + 1) * DH],
                                         start=(ltp == 0), stop=(ltp == lt))
                else:
                    nc.tensor.matmul(bps[:], self.ones[0:1, 0:PTQ], vm[:], start=True, stop=True)
                ob = self.kpool.tile([PTQ, DH], F32, tag="ob")
                nc.vector.tensor_scalar(ob[:], bps[:], ntop[:, lt:lt + 1], None, op0=AL.mult)
                stp = self.psum_s.tile([u, PTQ], F16, tag="psS")
                nc.tensor.matmul(stp[:], sel[:, lt, :], self.ident[0:PTQ, 0:PTQ],
                                 is_transpose=True, start=True, stop=True)
                selt = self.kpool.tile([u, PTQ], F16, tag="selt")
                nc.scalar.copy(selt[:], stp[:])
                ops_ = self.psum_m.tile([PTQ, DH], F32, tag="pm")
                nc.tensor.matmul(ops_[:], selt[:], upd[:], start=True, stop=True)
                olt = self.kpool.tile([PTQ, DH], F16, tag="olt")
                nc.vector.tensor_tensor(olt[:], ops_[:], ob[:], op=AL.add)
                otp = self.psum_s.tile([DH, PTQ], F16, tag="psS")
                nc.tensor.matmul(otp[:], olt[:], self.ident[0:PTQ, 0:PTQ],
                                 is_transpose=True, start=True, stop=True)
                nc.scalar.copy(OT[r0:r0 + DH, ti, lt * PTQ:(lt + 1) * PTQ], otp[:])
        return OT

    def attn_out_proj(self, aname, OT, Xres, LQ):
        """Wo projection + bias + residual add."""
        nc = self.nc
        bo = self.ld(aname + 'bo', [128, 4], F32, pool=self.cpool)

        def evict(ps, m, h0, nn, out):
            nc.vector.scalar_tensor_tensor(out[:, m, h0:h0 + nn], ps[:], bo[:, m:m + 1],
                                           Xres[:, m, h0:h0 + nn], op0=AL.add, op1=AL.add)
        out = self.rpool.tile([128, 4, LQ], F16, tag="resid")
        self.lin_dmaj(OT, aname + 'wo', LQ, evict=evict, out=out)
        return out


def _transpose_mm(nc, kb, out_ps, in_ap, ident):
    nc.tensor.matmul(out_ps, in_ap, ident, is_transpose=True, start=True, stop=True)


def build_program():
    nc = bacc.Bacc("TRN2", target_bir_lowering=False, debug=False, num_devices=8)
    consts = _host_constants()

    dram = {}

    def din(name, shape, dt=F32):
        dram[name] = nc.dram_tensor(name, list(shape), dt, kind="ExternalInput")

    # constants
    din('iota', [128, 720]); din('ident', [128, 128], F16)
    din('ident32', [128, 128]); din('ones', [128, 128], F16)
    din('ones32', [1, 128])
    for a, sp in ATTN_SPECS.items():
        PT = _pt(sp['LQ'])
        NQ = sp['LQ'] // PT
        din(f'cm_{a}', [PT, NQ, sp['LK']], F16)
        din(f'nm_{a}', [PT, NQ, sp['LK']], F16)
        din(f'lt_{a}', [PT, NQ, sp['LQ']], F16)
        if sp['masked']:
            din(f'ut_{a}', [PT, NQ, sp['LQ']], F16)
            din(f'ic_{a}', [PT, NQ, 1], F16)
    din('pos_e', [128, 4, SEQ], F16); din('pos_d', [128, 4, DEC])
    din('cwd2l', [128, 3, 3, D], F16); din('xdtl', [128, 3, DEC], F16)
    # weights
    for pref in ('e0', 'e1', 'ds', 'dc'):
        for nm in ('wq', 'wk', 'wv', 'wo'):
            din(pref + nm, [128, 4, D], F16)
        for nm in ('bq', 'bk', 'bo'):
            din(pref + nm, [128, 4])
        din(pref + 'bvr', [1, D], F16)
    din('dswql', [128, 4, D], F16)
    din('dswkl', [128, 4, D], F16)
    for pref in ('e0', 'e1', 'd'):
        din(pref + 'w1', [128, 4, DFF], F16)
        din(pref + 'w2', [128, 16, D], F16)
        din(pref + 'b1', [128, 16])
        din(pref + 'b2', [128, 4])
    for pref in ('e0n1', 'e0n2', 'e1n1', 'e1n2', 'encn', 'dn1', 'dn2', 'dn3', 'decn'):
        din(pref + 'g', [128, 4])
        din(pref + 'b', [128, 4])
    din('cwe', [128, 3, 3, D], F16); din('mwe', [MARK, D], F16)
    din('cwd2', [128, 3, 3, D], F16); din('mwd', [MARK, D], F16)
    din('cwd', [128, 4, 3, D], F16); din('bd', [128, 4])
    din('pw', [128, 4, CINP], F16); din('pb', [128, 3])
    # per-core inputs
    din('xet', [128, 3, SEQ])            # x_enc^T padded to 384 channels, f32
    din('xmet', [MARK, SEQ], F16)
    din('xdt', [128, 3, DEC], F16)       # x_dec^T padded, fp16
    din('xmdt', [MARK, DEC], F16)
    out_d = nc.dram_tensor('out', [128, 3, PRED], F32, kind="ExternalOutput")

    with tile.TileContext(nc) as tc:
        with ExitStack() as ctx:
            ctx.enter_context(nc.allow_low_precision(reason="deliberate fp16 kernel"))
            kb = KB(nc, tc, ctx, dram)
            stop = STOP_AFTER

            # ---- RevIN ----
            meanc = kb.cpool.tile([128, 3], F32, tag="meanc")
            sdc = kb.cpool.tile([128, 3], F32, tag="sdc")
            xn = kb.apool.tile([128, 3, SEQ], F16, tag="xnorm")
            for c in range(3):
                xch = kb.spool.tile([128, SEQ], F32, tag="xetc")
                nc.sync.dma_start(out=xch[:], in_=dram['xet'].ap()[:, c, :])
                sm = kb.kpool.tile([128, 1], F32, tag="mx")
                nc.vector.tensor_reduce(sm[:], xch[:], axis=AX.X, op=AL.add)
                nc.scalar.activation(meanc[:, c:c + 1], sm[:], AF.Copy, scale=1.0 / SEQ)
                xc = kb.spool.tile([128, SEQ], F32, tag="xetc")
                nc.vector.tensor_scalar(xc[:], xch[:], meanc[:, c:c + 1], None, op0=AL.subtract)
                var = kb.kpool.tile([128, 1], F32, tag="sm")
                scr2 = kb.spool.tile([128, SEQ], F32, tag="xetc")
                nc.vector.scalar_tensor_tensor(scr2[:], xc[:], 1.0 / SEQ, xc[:],
                                               op0=AL.mult, op1=AL.mult, accum_out=var[:])
                nc.scalar.activation(sdc[:, c:c + 1], var[:], AF.Sqrt, bias=kb.epsc[:], scale=1.0)
                rsd = kb.kpool.tile([128, 1], F32, tag="cgt")
                nc.vector.reciprocal(rsd[:], sdc[:, c:c + 1])
                nc.vector.tensor_scalar(xn[:, c, :], xc[:], rsd[:], None, op0=AL.mult)

            # ---- encoder embed ----
            xmet = kb.ld('xmet', [MARK, SEQ], F16, pool=kb.cpool)
            X, _ = kb.embed(xn, xmet, 'cwe', 'mwe', 'pos_e', SEQ, 3)
            kb.dump_dmaj('X0', X, 4, SEQ)
            if stop != 'embed':
                # ---- encoder layer 0 ----
                OT = kb.attention('e0', X, X, SEQ, SEQ)
                R = kb.attn_out_proj('e0', OT, X, SEQ)
                X = kb.layernorm(R, 'e0n1g', 'e0n1b', SEQ, "resid")
                R = kb.ffn(X, 'e0', SEQ)
                X = kb.layernorm(R, 'e0n2g', 'e0n2b', SEQ, "resid")
                kb.dump_dmaj('X_enc0', X, 4, SEQ)
            if stop not in ('embed', 'enc0'):
                # ---- distill conv ----
                bd = kb.ld('bd', [128, 4], F32, pool=kb.cpool)
                xpad = kb.apool.tile([128, 4, SEQ + 2], F16, tag="xa")
                for c in range(4):
                    nc.vector.tensor_copy(xpad[:, c, 1:SEQ + 1], X[:, c, :])
                    nc.vector.tensor_copy(xpad[:, c, 0:1], X[:, c, SEQ - 1:SEQ])
                    nc.vector.tensor_copy(xpad[:, c, SEQ + 1:SEQ + 2], X[:, c, 0:1])
                cw = kb.bigp.tile([128, 4, 3, D], F16, tag="big")
                nc.sync.dma_start(out=cw[:], in_=dram['cwd'].ap())
                X2 = kb.rpool.tile([128, 4, 360], F16, tag="resid")
                for m in range(4):
                    celu = kb.spool.tile([128, SEQ], F16, tag="celu")
                    for (h0, nn) in _halves(SEQ):
                        ps = kb.psum_n.tile([128, nn], F32, tag="pn")
                        first = True
                        for k in range(3):
                            for c in range(4):
                                nc.tensor.matmul(ps[:], cw[:, c, k, m * 128:(m + 1) * 128],
                                                 xpad[:, c, k + h0:k + h0 + nn],
                                                 start=first, stop=(k == 2 and c == 3))
                                first = False
                        nc.scalar.activation(celu[:, h0:h0 + nn], ps[:], AF.Identity,
                                             bias=bd[:, m:m + 1], scale=1.0)
                    # ELU
                    t1 = kb.spool.tile([128, SEQ], F16, tag="scr")
                    nc.vector.tensor_scalar(t1[:], celu[:], 0.0, None, op0=AL.min)
                    t2 = kb.spool.tile([128, SEQ], F16, tag="scr")
                    nc.scalar.activation(t2[:], t1[:], AF.Exp)
                    t3 = kb.spool.tile([128, SEQ], F16, tag="celu2")
                    nc.scalar.activation(t3[:], celu[:], AF.Relu)
                    el = kb.spool.tile([128, SEQ], F16, tag="celu")
                    nc.vector.scalar_tensor_tensor(el[:], t3[:], -1.0, t2[:], op0=AL.add, op1=AL.add)
                    # maxpool k=3 s=2 pad=1
                    e3 = el[:].rearrange("p (n two) -> p n two", two=2)
                    nc.vector.tensor_tensor(X2[:, m, :], e3[:, :, 0], e3[:, :, 1], op=AL.max)
                    nc.vector.tensor_tensor(X2[:, m, 1:360], X2[:, m, 1:360],
                                            el[:, 1:719].rearrange("p (n two) -> p n two", two=2)[:, :, 0],
                                            op=AL.max)
                X = X2
                kb.dump_dmaj('X_dist', X, 4, 360)
            if stop not in ('embed', 'enc0', 'distill'):
                # ---- encoder layer 1 ----
                OT = kb.attention('e1', X, X, 360, 360)
                R = kb.attn_out_proj('e1', OT, X, 360)
                X = kb.layernorm(R, 'e1n1g', 'e1n1b', 360, "resid")
                R = kb.ffn(X, 'e1', 360)
                X = kb.layernorm(R, 'e1n2g', 'e1n2b', 360, "resid")
                cross = kb.layernorm(X, 'encng', 'encnb', 360, "cross")
                kb.dump_dmaj('cross', cross, 4, 360)
            if stop in ('dembed', 'ds', 'dc', 'all'):
                # ---- decoder embed ----
                xdt = kb.ld('xdt', [128, 3, DEC], F16, pool=kb.apool, tag="xnorm")
                xdtl = kb.ld('xdtl', [128, 3, DEC], F16, pool=kb.apool, tag="xnorml")
                xmdt = kb.ld('xmdt', [MARK, DEC], F16, pool=kb.cpool)
                Y, Ylo = kb.embed(xdt, xmdt, 'cwd2', 'mwd', 'pos_d', DEC, 3,
                                  xtl=xdtl, want_lo=True)
                kb.dump_dmaj('Y0', Y, 4, DEC)
            if stop in ('ds', 'dc', 'all'):
                OT = kb.attention('ds', Y, Y, DEC, DEC, XQlo=Ylo, XKVlo=Ylo)
                R = kb.attn_out_proj('ds', OT, Y, DEC)
                Y = kb.layernorm(R, 'dn1g', 'dn1b', DEC, "resid")
                kb.dump_dmaj('Y_ds', Y, 4, DEC)
            if stop in ('dc', 'all'):
                OT = kb.attention('dc', Y, cross, DEC, 360)
                R = kb.attn_out_proj('dc', OT, Y, DEC)
                Y = kb.layernorm(R, 'dn2g', 'dn2b', DEC, "resid")
                R = kb.ffn(Y, 'd', DEC)
                Y = kb.layernorm(R, 'dn3g', 'dn3b', DEC, "resid")
                Y = kb.layernorm(Y, 'decng', 'decnb', DEC, "resid")
                kb.dump_dmaj('Y_out', Y, 4, DEC)
            if stop == 'all':
                # ---- projection + de-norm, first 336 cols only ----
                pb = kb.ld('pb', [128, 3], F32, pool=kb.cpool)
                pwd = dram['pw']
                outsb = kb.spool.tile([128, 3, PRED], F32, tag="outsb", bufs=1)
                wt = kb.wpool.tile([128, 4, CINP], F16, tag="wstream")
                nc.sync.dma_start(out=wt[:], in_=pwd.ap())
                for m in range(3):
                    ps = kb.psum_n.tile([128, PRED], F32, tag="pn")
                    for kc in range(4):
                        nc.tensor.matmul(ps[:], wt[:, kc, m * 128:(m + 1) * 128],
                                         Y[:, kc, 0:PRED], start=(kc == 0), stop=(kc == 3))
                    t = kb.spool.tile([128, PRED], F32, tag="psb")
                    nc.vector.tensor_scalar(t[:], ps[:], pb[:, m:m + 1], None, op0=AL.add)
                    nc.vector.tensor_scalar(outsb[:, m, :], t[:], sdc[:, m:m + 1],
                                            meanc[:, m:m + 1], op0=AL.mult, op1=AL.add)
                    nc.sync.dma_start(out=out_d.ap()[:, m, :], in_=outsb[:, m, :])

    nc.compile()
    return nc


_PROG = None


def _in_maps(x_enc, x_mark_enc, x_dec, x_mark_dec, params):
    consts = _host_constants()
    w = _pack_params(params)
    x_enc = np.asarray(x_enc, np.float32)
    x_me = np.asarray(x_mark_enc, np.float32)
    x_dec = np.asarray(x_dec, np.float32)
    x_md = np.asarray(x_mark_dec, np.float32)
    base = dict(consts)
    base.update(w)
    in_maps = []
    for b in range(B):
        m = dict(base)
        xe = np.zeros((CINP, SEQ), np.float32)
        xe[:CIN] = x_enc[b].T
        m['xet'] = np.ascontiguousarray(xe.reshape(3, 128, SEQ).transpose(1, 0, 2))
        m['xmet'] = _f16(x_me[b].T)
        xd = np.zeros((CINP, DEC), np.float32)
        xd[:CIN] = x_dec[b].T
        xd = np.ascontiguousarray(xd.reshape(3, 128, DEC).transpose(1, 0, 2))
        xdh = xd.astype(np.float16)
        m['xdt'] = xdh
        m['xdtl'] = (xd - xdh.astype(np.float32)).astype(np.float16)
        m['xmdt'] = _f16(x_md[b].T)
        in_maps.append(m)
    return in_maps


def _unpack_out(results):
    outs = []
    for b in range(B):
        o = results[b]['out']              # [128, 3, 336]
        o = o.transpose(1, 0, 2).reshape(CINP, PRED)[:COUT]   # [321, 336]
        outs.append(o.T)                    # [336, 321]
    return np.stack(outs, 0).astype(np.float32)


def get_program():
    global _PROG
    if _PROG is None:
        _PROG = build_program()
    return _PROG


def kernel(x_enc, x_mark_enc, x_dec, x_mark_dec, params):
    nc = get_program()
    in_maps = _in_maps(x_enc, x_mark_enc, x_dec, x_mark_dec, params)
    res = bass_utils.run_bass_kernel_spmd(nc, in_maps, core_ids=list(range(B)))
    return _unpack_out(res.results)


if __name__ == '__main__':
    pass
